# revision 8
# baseline (speedup 1.0000x reference)
"""Tensor-parallel causal self-attention kernel for 8 Trainium2 NeuronCores.

Problem: B=2, T=2048, C=2048, H=16 heads x hd=128, rotate-half RoPE,
causal softmax, out projection. Returns (y, k, v) with k, v post-RoPE in
[B, H, T, hd] layout.

Sharding: TP-4 over heads x DP-2 over batch. Core c handles batch c//4 and
global heads {4*(c%4) .. 4*(c%4)+3}. Each core computes QKV for its heads
(fp32r matmuls, [out_dim, T] transposed layout), RoPE, causal attention in
S^T=[kv,q] layout, then per-q-block AllGathers within each batch group of 4
cores exchange attention outputs so every core computes the full
w_out^T @ attT for its own 512-query slice of yT (selected with a
cc_rank-driven dynamic DMA offset). The output projection is split per
head-pair pass so half of it overlaps the second pass's attention.
"""

import os
import sys

sys.path.insert(0, "/opt/trn_rl_repo")

import numpy as np

import concourse.bass as bass
import concourse.mybir as mybir
import concourse.tile as tile
from concourse import bacc
from concourse.bass_utils import run_bass_kernel_spmd

F32 = mybir.dt.float32
F32R = mybir.dt.float32r

N_CORES = 8
B, T, C = 2, 2048, 2048
H, HD = 16, 128
ROPE_BASE = 10000.0

GROUP = 4            # cores per batch group (TP degree)
HPC = H // GROUP     # heads per core = 4
NPASS = 2            # head-pairs per core
QB = 512             # q-block / nt-slice width
NQB = T // QB        # 4
NCH = C // 128       # 16 contraction chunks
NKV = T // 128       # 16 kv chunks
SCALE = 1.0 / float(np.sqrt(HD))

_CACHED = {}


def _phase_b(nc, tc, p, qkvp, scrp, cos_sb, sin_sb, wproj_r, xT_r, kT_out, v_out):
    """QKV projection + fused RoPE for head pair p. Returns (qkT, v_sb)."""
    qkT = qkvp.tile([128, 2, 2, T], F32R, tag="qkT", name="qkT")  # [d, hh, q/k, t]
    v_sb = qkvp.tile([128, 2, NKV, HD], F32R, tag="v", name="v_sb")

    with (
        tc.tile_pool(name=f"xw{p}", bufs=2) as xwp,
        tc.tile_pool(name=f"wc{p}", bufs=1) as wcp,
        tc.tile_pool(name=f"psB{p}", bufs=3, space="PSUM") as psB,
    ):
        wc = wcp.tile([128, NCH, 6, 128], F32R, tag="wc", name="wc")
        for ci in range(NCH):
            nc.sync.dma_start(out=wc[:, ci], in_=wproj_r[:, ci, p])

        for nt in range(NQB):
            xh = [xwp.tile([128, 8, QB], F32R, tag=f"xh{i}", name=f"xh{i}")
                  for i in range(2)]
            for i in range(2):
                nc.sync.dma_start(
                    out=xh[i][:],
                    in_=xT_r[:, 8 * i:8 * (i + 1), nt * QB:(nt + 1) * QB],
                )

            # q/k outputs: 4 targets (hh x q/k), accumulate 16 chunks
            for hh in range(2):
                for qk in range(2):
                    ps = psB.tile([128, QB], F32, tag="qk", name="ps_qk")
                    for ci in range(NCH):
                        nc.tensor.matmul(
                            ps[:],
                            wc[:, ci, 2 * qk + hh],
                            xh[ci // 8][:, ci % 8],
                            start=(ci == 0),
                            stop=(ci == NCH - 1),
                        )
                    # RoPE fused with PSUM->SBUF: out = res*cos + rot(res)*sin'
                    dst = qkT[:, hh, qk, nt * QB:(nt + 1) * QB]
                    cs = cos_sb[:, nt * QB:(nt + 1) * QB]
                    sn = sin_sb[:, nt * QB:(nt + 1) * QB]
                    rot = scrp.tile([128, QB], F32, tag="rot", name="rot")
                    tmp = scrp.tile([128, QB], F32, tag="tmp", name="tmp")
                    nc.vector.tensor_copy(out=rot[0:64, :], in_=ps[64:128, :])
                    nc.vector.tensor_copy(out=rot[64:128, :], in_=ps[0:64, :])
                    nc.vector.tensor_tensor(
                        out=dst, in0=ps[:], in1=cs, op=mybir.AluOpType.mult)
                    nc.vector.tensor_tensor(
                        out=tmp[:], in0=rot[:], in1=sn, op=mybir.AluOpType.mult)
                    nc.vector.tensor_tensor(
                        out=dst, in0=dst, in1=tmp[:], op=mybir.AluOpType.add)

            # v outputs: natural layout, t-tiles 4nt..4nt+3
            for tt in range(4):
                psv = psB.tile([128, 256], F32, tag="v", name="ps_v")
                for ci in range(NCH):
                    nc.tensor.matmul(
                        psv[:],
                        xh[ci // 8][:, ci % 8, tt * 128:(tt + 1) * 128],
                        wc[:, ci, 4:6].rearrange("p j d -> p (j d)"),
                        start=(ci == 0),
                        stop=(ci == NCH - 1),
                    )
                for hh in range(2):
                    nc.scalar.copy(
                        out=v_sb[:, hh, 4 * nt + tt, :],
                        in_=psv[:, hh * HD:(hh + 1) * HD],
                    )

        # write k, v outputs for this pass's heads
        for hh in range(2):
            lh = 2 * p + hh
            nc.sync.dma_start(out=kT_out[lh], in_=qkT[:, hh, 1, :])
            nc.sync.dma_start(
                out=v_out[lh].rearrange("c pp d -> pp c d"),
                in_=v_sb[:, hh],
            )
    return qkT, v_sb


def _phase_c(nc, tc, p, qkT, v_sb, scrp, ones_sb, masks, ag_in, ag_out, groups):
    """Causal attention in S^T layout + per-q-block AllGathers."""
    with (
        tc.tile_pool(name=f"pt{p}", bufs=4) as ptp,
        tc.tile_pool(name=f"at{p}", bufs=3) as atp,
        tc.tile_pool(name=f"mk{p}", bufs=1) as mkp,
        tc.tile_pool(name=f"psS{p}", bufs=3, space="PSUM") as psS,
        tc.tile_pool(name=f"psO{p}", bufs=2, space="PSUM") as psO,
        tc.tile_pool(name=f"psD{p}", bufs=2, space="PSUM") as psD,
    ):
        mask_sb = mkp.tile([128, 4, QB], F32R, tag="mask", name="mask_sb")
        nc.sync.dma_start(out=mask_sb[:], in_=masks[:])
        for qb in range(NQB):
            for hh in range(2):
                nj = 4 * qb + 4  # kv chunks for this q block
                ps_o = psO.tile([128, QB], F32, tag="o", name="ps_o")
                den = atp.tile([128, QB], F32R, tag="den", name="den")
                qrhs = qkT[:, hh, 0, qb * QB:(qb + 1) * QB]
                prev_pt = None
                for j in range(nj):
                    ps_s = psS.tile([128, QB], F32, tag="s", name="ps_s")
                    nc.tensor.matmul(
                        ps_s[:],
                        qkT[:, hh, 1, j * 128:(j + 1) * 128],
                        qrhs,
                        start=True, stop=True,
                    )
                    pt = ptp.tile([128, QB], F32R, tag="pt", name="pt")
                    nc.scalar.activation(
                        pt[:], ps_s[:],
                        mybir.ActivationFunctionType.Exp,
                        scale=SCALE,
                    )
                    if j >= 4 * qb:  # diagonal chunk: causal mask
                        nc.vector.tensor_tensor(
                            out=pt[:], in0=pt[:],
                            in1=mask_sb[:, j - 4 * qb],
                            op=mybir.AluOpType.mult,
                        )
                    # softmax denominator accumulates on DVE (PE stays on
                    # scores/PV matmuls)
                    if j == 1:
                        nc.vector.tensor_tensor(
                            out=den[:], in0=prev_pt[:], in1=pt[:],
                            op=mybir.AluOpType.add)
                    elif j > 1:
                        nc.vector.tensor_tensor(
                            out=den[:], in0=den[:], in1=pt[:],
                            op=mybir.AluOpType.add)
                    prev_pt = pt
                    nc.tensor.matmul(
                        ps_o[:], v_sb[:, hh, j, :], pt[:],
                        start=(j == 0), stop=(j == nj - 1),
                    )
                ps_d = psD.tile([1, QB], F32, tag="d", name="ps_d")
                nc.tensor.matmul(
                    ps_d[:], ones_sb[:], den[:], start=True, stop=True)
                recip = scrp.tile([1, QB], F32, tag="recip", name="recip")
                nc.vector.reciprocal(recip[:], ps_d[:])
                bc_sb = atp.tile([128, QB], F32, tag="bcs", name="bc_sb")
                nc.gpsimd.partition_broadcast(bc_sb[:], recip[:])
                att = atp.tile([128, QB], F32R, tag="att", name="att")
                nc.vector.tensor_tensor(
                    out=att[:], in0=ps_o[:], in1=bc_sb[:],
                    op=mybir.AluOpType.mult,
                )
                nc.sync.dma_start(out=ag_in[p][qb, hh], in_=att[:])
            # per-q-block AllGather: fires as soon as both heads' attT
            # for this q block are written
            nc.gpsimd.collective_compute(
                "AllGather", mybir.AluOpType.bypass,
                replica_groups=groups,
                ins=[ag_in[p][qb].opt()],
                outs=[ag_out[p][qb].opt()],
            )


def _phase_e(nc, tc, p, ag_out, wout, y_acc_dram, y_out, groups):
    """Half output projection for pass p's heads (4 srcs x 2 heads)."""
    with (
        tc.tile_pool(name=f"ysb{p}", bufs=3) as ysbp,
        tc.tile_pool(name=f"a2asb{p}", bufs=1) as a2ap,
        tc.tile_pool(name=f"wo{p}", bufs=8) as wop,
        tc.tile_pool(name=f"psY{p}", bufs=3, space="PSUM") as psY,
    ):
        a2a_sb = a2ap.tile([128, GROUP, 2, QB], F32R, tag="a2asb", name="a2a_sb")
        rank = nc.sync.cc_rank(groups)
        # this core's QB-wide q slice of the gathered attT lives entirely in
        # AG chunk qb == rank; select it with a dynamic outer-dim offset
        agf = ag_out[p].rearrange("qb s h d q -> (qb s h) d q")
        for s in range(GROUP):
            for hh in range(2):
                nc.sync.dma_start(
                    out=a2a_sb[:, s, hh, :],
                    in_=agf[bass.ds(rank * (GROUP * 2) + (2 * s + hh), 1)],
                )
        for ct in range(NCH):
            ps_y = psY.tile([128, QB], F32, tag="y", name="ps_y")
            for s in range(GROUP):
                for hh in range(2):
                    hglob = 4 * s + 2 * p + hh
                    wt = wop.tile([128, 128], F32R, tag="wt", name="wt")
                    nc.sync.dma_start(
                        out=wt[:], in_=wout[hglob, :, ct * 128:(ct + 1) * 128])
                    nc.tensor.matmul(
                        ps_y[:], wt[:], a2a_sb[:, s, hh, :],
                        start=(s == 0 and hh == 0),
                        stop=(s == GROUP - 1 and hh == 1),
                    )
            y_sb = ysbp.tile([128, QB], F32, tag="ysb", name="ysb")
            if p == 0:
                nc.scalar.copy(out=y_sb[:], in_=ps_y[:])
                nc.sync.dma_start(out=y_acc_dram[ct], in_=y_sb[:])
            else:
                part = ysbp.tile([128, QB], F32, tag="part", name="part")
                nc.sync.dma_start(out=part[:], in_=y_acc_dram[ct])
                nc.vector.tensor_tensor(
                    out=y_sb[:], in0=ps_y[:], in1=part[:],
                    op=mybir.AluOpType.add)
                nc.sync.dma_start(
                    out=y_out[ct * 128:(ct + 1) * 128, :], in_=y_sb[:])


def build_nc():
    nc = bacc.Bacc(None, num_devices=N_CORES)

    # ---- DRAM parameters (per-core data) ----
    xT = nc.declare_dram_parameter("xT", [C, T], F32R, isOutput=False)
    wproj = nc.declare_dram_parameter("wproj", [C, NPASS, 6, 128], F32R, isOutput=False)
    wout = nc.declare_dram_parameter("wout", [H, HD, C], F32R, isOutput=False)
    cos2 = nc.declare_dram_parameter("cos2", [128, T], F32R, isOutput=False)
    sins = nc.declare_dram_parameter("sins", [128, T], F32R, isOutput=False)
    masks = nc.declare_dram_parameter("masks", [128, 4, QB], F32R, isOutput=False)
    ones = nc.declare_dram_parameter("ones", [128, 1], F32R, isOutput=False)

    kT_out = nc.declare_dram_parameter("kT_out", [HPC, 128, T], F32R, isOutput=True)
    v_out = nc.declare_dram_parameter("v_out", [HPC, NKV, 128, HD], F32R, isOutput=True)
    y_out = nc.declare_dram_parameter("y_out", [C, QB], F32, isOutput=True)

    groups = [[0, 1, 2, 3], [4, 5, 6, 7]]

    with tile.TileContext(nc) as tc:
        with (
            tc.tile_pool(name="const", bufs=1) as constp,
            tc.tile_pool(name="dram", bufs=1, space="DRAM") as dramp,
        ):
            cos_sb = constp.tile([128, T], F32R, tag="cos", name="cos_sb")
            sin_sb = constp.tile([128, T], F32R, tag="sin", name="sin_sb")
            ones_sb = constp.tile([128, 1], F32R, tag="ones", name="ones_sb")
            nc.sync.dma_start(out=cos_sb[:], in_=cos2[:])
            nc.sync.dma_start(out=sin_sb[:], in_=sins[:])
            nc.sync.dma_start(out=ones_sb[:], in_=ones[:])

            # per (pass, q-block) AllGather buffers (one contiguous tile
            # per pass so phase E can select chunk qb==rank dynamically)
            ag_in = [dramp.tile([NQB, 2, 128, QB], F32R, name=f"ag_in{p}")
                     for p in range(NPASS)]
            ag_out = [dramp.tile([NQB, GROUP, 2, 128, QB], F32R, name=f"ag_out{p}")
                      for p in range(NPASS)]
            y_acc_dram = dramp.tile([NCH, 128, QB], F32, name="y_acc")

            xT_r = xT.rearrange("(ci p) t -> p ci t", p=128)
            wproj_r = wproj.rearrange("(ci p) s j d -> p ci s j d", p=128)

            for p in range(NPASS):
                with (
                    tc.tile_pool(name=f"qkv{p}", bufs=1) as qkvp,
                    tc.tile_pool(name=f"scr{p}", bufs=2) as scrp,
                ):
                    qkT, v_sb = _phase_b(
                        nc, tc, p, qkvp, scrp, cos_sb, sin_sb,
                        wproj_r, xT_r, kT_out, v_out)
                    if p == 1:
                        # pass-0 output projection: AllGather #0 chunks have
                        # long since landed; runs on a warm PE before the
                        # pass-1 attention stream
                        _phase_e(nc, tc, 0, ag_out, wout, y_acc_dram,
                                 y_out, groups)
                    _phase_c(nc, tc, p, qkT, v_sb, scrp, ones_sb, masks,
                             ag_in, ag_out, groups)

            _phase_e(nc, tc, 1, ag_out, wout, y_acc_dram, y_out, groups)

    nc.compile()
    return nc


def _host_prep(x, w_qkv, w_out, start_pos):
    x = np.asarray(x, dtype=np.float32)
    w_qkv = np.asarray(w_qkv, dtype=np.float32)
    w_out = np.asarray(w_out, dtype=np.float32)
    sp = int(start_pos)

    xT = [np.ascontiguousarray(x[b].T) for b in range(B)]

    # RoPE tables in [hd, T] layout (halves stacked), sin with baked sign
    inv_freq = 1.0 / (ROPE_BASE ** (np.arange(0, HD, 2, dtype=np.float32) / HD))
    pos = (sp + np.arange(T)).astype(np.float32)
    ang = np.outer(pos, inv_freq)          # [T, 64]
    cosv = np.cos(ang).T.astype(np.float32)  # [64, T]
    sinv = np.sin(ang).T.astype(np.float32)
    cos2 = np.ascontiguousarray(np.concatenate([cosv, cosv], 0))    # [128, T]
    sins = np.ascontiguousarray(np.concatenate([-sinv, sinv], 0))   # [128, T]

    # causal 0/1 masks for the 4 diagonal chunk offsets: [kv,(r),q]
    kv = np.arange(128)[:, None]
    q = np.arange(QB)[None, :]
    masks = np.stack(
        [(128 * r + kv <= q).astype(np.float32) for r in range(4)], axis=1)

    ones = np.ones((128, 1), np.float32)

    # per-core weight slices: wproj[c] = [C, pass, 6, 128]
    wq = w_qkv[:, 0 * C:1 * C].reshape(C, H, HD)
    wk = w_qkv[:, 1 * C:2 * C].reshape(C, H, HD)
    wv = w_qkv[:, 2 * C:3 * C].reshape(C, H, HD)
    wproj_cores = []
    for hg in range(GROUP):
        parts = []
        for p in range(NPASS):
            h0, h1 = 4 * hg + 2 * p, 4 * hg + 2 * p + 1
            parts.append(np.stack(
                [wq[:, h0], wq[:, h1], wk[:, h0], wk[:, h1],
                 wv[:, h0], wv[:, h1]], axis=1))   # [C, 6, 128]
        wproj_cores.append(np.ascontiguousarray(
            np.stack(parts, axis=1)))              # [C, 2, 6, 128]

    wout_r = np.ascontiguousarray(w_out.reshape(H, HD, C))

    in_maps = []
    for c in range(N_CORES):
        b, hg = c // GROUP, c % GROUP
        in_maps.append({
            "xT": xT[b],
            "wproj": wproj_cores[hg],
            "wout": wout_r,
            "cos2": cos2,
            "sins": sins,
            "masks": masks,
            "ones": ones,
        })
    return in_maps


def _assemble(results):
    y = np.empty((B, T, C), np.float32)
    k = np.empty((B, H, T, HD), np.float32)
    v = np.empty((B, H, T, HD), np.float32)
    for c in range(N_CORES):
        b, hg = c // GROUP, c % GROUP
        r = results[c]
        y[b, QB * hg:QB * (hg + 1), :] = r["y_out"].T
        for lh in range(HPC):
            hglob = 4 * hg + lh
            k[b, hglob] = r["kT_out"][lh].T
            v[b, hglob] = r["v_out"][lh].reshape(T, HD)
    return y, k, v


def run(x, w_qkv, w_out, start_pos, **spmd_kwargs):
    if "nc" not in _CACHED:
        _CACHED["nc"] = build_nc()
    nc = _CACHED["nc"]
    in_maps = _host_prep(x, w_qkv, w_out, start_pos)
    res = run_bass_kernel_spmd(nc, in_maps, list(range(N_CORES)), **spmd_kwargs)
    return _assemble(res.results), res


def kernel(x, w_qkv, w_out, start_pos):
    (y, k, v), _ = run(x, w_qkv, w_out, start_pos)
    return y, k, v


# revision 14
# speedup vs baseline: 1.0113x; 1.0113x over previous
"""Tensor-parallel causal self-attention kernel for 8 Trainium2 NeuronCores.

Problem: B=2, T=2048, C=2048, H=16 heads x hd=128, rotate-half RoPE,
causal softmax, out projection. Returns (y, k, v) with k, v post-RoPE in
[B, H, T, hd] layout.

Sharding: TP-4 over heads x DP-2 over batch. Core c handles batch c//4 and
global heads {4*(c%4) .. 4*(c%4)+3}. Each core computes QKV for its heads
(fp32r matmuls, [out_dim, T] transposed layout), RoPE, causal attention in
S^T=[kv,q] layout, then per-q-block AllGathers within each batch group of 4
cores exchange attention outputs so every core computes the full
w_out^T @ attT for its own 512-query slice of yT (selected with a
cc_rank-driven dynamic DMA offset). The output projection is split per
head-pair pass so half of it overlaps the second pass's attention.
"""

import os
import sys

sys.path.insert(0, "/opt/trn_rl_repo")

import numpy as np

import concourse.bass as bass
import concourse.mybir as mybir
import concourse.tile as tile
from concourse import bacc
from concourse.bass_utils import run_bass_kernel_spmd

F32 = mybir.dt.float32
F32R = mybir.dt.float32r

N_CORES = 8
B, T, C = 2, 2048, 2048
H, HD = 16, 128
ROPE_BASE = 10000.0

GROUP = 4            # cores per batch group (TP degree)
HPC = H // GROUP     # heads per core = 4
NPASS = 2            # head-pairs per core
QB = 512             # q-block / nt-slice width
NQB = T // QB        # 4
NCH = C // 128       # 16 contraction chunks
NKV = T // 128       # 16 kv chunks
SCALE = 1.0 / float(np.sqrt(HD))

_CACHED = {}


def _phase_b(nc, tc, p, qkvp, scrp, cos_sb, sin_sb, wproj_r, xT_r, kT_out, v_out):
    """QKV projection + fused RoPE for head pair p. Returns (qkT, v_sb)."""
    qkT = qkvp.tile([128, 2, 2, T], F32R, tag="qkT", name="qkT")  # [d, hh, q/k, t]
    v_sb = qkvp.tile([128, 2, NKV, HD], F32R, tag="v", name="v_sb")

    with (
        tc.tile_pool(name=f"xw{p}", bufs=2) as xwp,
        tc.tile_pool(name=f"wc{p}", bufs=1) as wcp,
        tc.tile_pool(name=f"psB{p}", bufs=3, space="PSUM") as psB,
    ):
        wc = wcp.tile([128, NCH, 6, 128], F32R, tag="wc", name="wc")
        for ci in range(NCH):
            nc.sync.dma_start(out=wc[:, ci], in_=wproj_r[:, ci, p])

        for nt in range(NQB):
            xh = [xwp.tile([128, 8, QB], F32R, tag=f"xh{i}", name=f"xh{i}")
                  for i in range(2)]
            for i in range(2):
                nc.sync.dma_start(
                    out=xh[i][:],
                    in_=xT_r[:, 8 * i:8 * (i + 1), nt * QB:(nt + 1) * QB],
                )

            # q/k outputs: 4 targets (hh x q/k), accumulate 16 chunks
            for hh in range(2):
                for qk in range(2):
                    ps = psB.tile([128, QB], F32, tag="qk", name="ps_qk")
                    for ci in range(NCH):
                        nc.tensor.matmul(
                            ps[:],
                            wc[:, ci, 2 * qk + hh],
                            xh[ci // 8][:, ci % 8],
                            start=(ci == 0),
                            stop=(ci == NCH - 1),
                        )
                    # RoPE fused with PSUM->SBUF: out = res*cos + rot(res)*sin'
                    dst = qkT[:, hh, qk, nt * QB:(nt + 1) * QB]
                    cs = cos_sb[:, nt * QB:(nt + 1) * QB]
                    sn = sin_sb[:, nt * QB:(nt + 1) * QB]
                    rot = scrp.tile([128, QB], F32, tag="rot", name="rot")
                    tmp = scrp.tile([128, QB], F32, tag="tmp", name="tmp")
                    nc.vector.tensor_copy(out=rot[0:64, :], in_=ps[64:128, :])
                    nc.vector.tensor_copy(out=rot[64:128, :], in_=ps[0:64, :])
                    nc.vector.tensor_tensor(
                        out=dst, in0=ps[:], in1=cs, op=mybir.AluOpType.mult)
                    nc.vector.tensor_tensor(
                        out=tmp[:], in0=rot[:], in1=sn, op=mybir.AluOpType.mult)
                    nc.vector.tensor_tensor(
                        out=dst, in0=dst, in1=tmp[:], op=mybir.AluOpType.add)

            # v outputs: natural layout, t-tiles 4nt..4nt+3
            for tt in range(4):
                psv = psB.tile([128, 256], F32, tag="v", name="ps_v")
                for ci in range(NCH):
                    nc.tensor.matmul(
                        psv[:],
                        xh[ci // 8][:, ci % 8, tt * 128:(tt + 1) * 128],
                        wc[:, ci, 4:6].rearrange("p j d -> p (j d)"),
                        start=(ci == 0),
                        stop=(ci == NCH - 1),
                    )
                for hh in range(2):
                    nc.vector.tensor_copy(
                        out=v_sb[:, hh, 4 * nt + tt, :],
                        in_=psv[:, hh * HD:(hh + 1) * HD],
                    )

        # write k, v outputs for this pass's heads
        for hh in range(2):
            lh = 2 * p + hh
            nc.sync.dma_start(out=kT_out[lh], in_=qkT[:, hh, 1, :])
            nc.sync.dma_start(
                out=v_out[lh].rearrange("c pp d -> pp c d"),
                in_=v_sb[:, hh],
            )
    return qkT, v_sb


def _phase_c(nc, tc, p, qkT, v_sb, scrp, ones_sb, masks, ag_in, ag_out, groups):
    """Causal attention in S^T layout + per-q-block AllGathers."""
    with (
        tc.tile_pool(name=f"pt{p}", bufs=4) as ptp,
        tc.tile_pool(name=f"at{p}", bufs=3) as atp,
        tc.tile_pool(name=f"mk{p}", bufs=1) as mkp,
        tc.tile_pool(name=f"psS{p}", bufs=3, space="PSUM") as psS,
        tc.tile_pool(name=f"psO{p}", bufs=2, space="PSUM") as psO,
        tc.tile_pool(name=f"psD{p}", bufs=2, space="PSUM") as psD,
    ):
        mask_sb = mkp.tile([128, 4, QB], F32R, tag="mask", name="mask_sb")
        nc.sync.dma_start(out=mask_sb[:], in_=masks[:])
        for qb in range(NQB):
            for hh in range(2):
                nj = 4 * qb + 4  # kv chunks for this q block
                ps_o = psO.tile([128, QB], F32, tag="o", name="ps_o")
                den = atp.tile([128, QB], F32R, tag="den", name="den")
                qrhs = qkT[:, hh, 0, qb * QB:(qb + 1) * QB]
                pts = []
                for j in range(nj):
                    ps_s = psS.tile([128, QB], F32, tag="s", name="ps_s")
                    nc.tensor.matmul(
                        ps_s[:],
                        qkT[:, hh, 1, j * 128:(j + 1) * 128],
                        qrhs,
                        start=True, stop=True,
                    )
                    pt = ptp.tile([128, QB], F32R, tag="pt", name="pt")
                    nc.scalar.activation(
                        pt[:], ps_s[:],
                        mybir.ActivationFunctionType.Exp,
                        scale=SCALE,
                    )
                    if j >= 4 * qb:  # diagonal chunk: causal mask
                        nc.vector.tensor_tensor(
                            out=pt[:], in0=pt[:],
                            in1=mask_sb[:, j - 4 * qb],
                            op=mybir.AluOpType.mult,
                        )
                    # softmax denominator accumulates on DVE (PE stays on
                    # scores/PV matmuls)
                    if j == 1:
                        nc.vector.tensor_tensor(
                            out=den[:], in0=pts[0][:], in1=pt[:],
                            op=mybir.AluOpType.add)
                    elif j > 1:
                        nc.vector.tensor_tensor(
                            out=den[:], in0=den[:], in1=pt[:],
                            op=mybir.AluOpType.add)
                    pts.append(pt)
                    # PV matmul for the PREVIOUS chunk: by the time the PE
                    # finishes s(j), exp(j-1) has drained, so the PE never
                    # stalls on the Activation engine
                    if j > 0:
                        nc.tensor.matmul(
                            ps_o[:], v_sb[:, hh, j - 1, :], pts[j - 1][:],
                            start=(j - 1 == 0), stop=False,
                        )
                nc.tensor.matmul(
                    ps_o[:], v_sb[:, hh, nj - 1, :], pts[nj - 1][:],
                    start=False, stop=True,
                )
                ps_d = psD.tile([1, QB], F32, tag="d", name="ps_d")
                nc.tensor.matmul(
                    ps_d[:], ones_sb[:], den[:], start=True, stop=True)
                recip = scrp.tile([1, QB], F32, tag="recip", name="recip")
                nc.vector.reciprocal(recip[:], ps_d[:])
                bc_sb = atp.tile([128, QB], F32, tag="bcs", name="bc_sb")
                nc.gpsimd.partition_broadcast(bc_sb[:], recip[:])
                att = atp.tile([128, QB], F32R, tag="att", name="att")
                nc.vector.tensor_tensor(
                    out=att[:], in0=ps_o[:], in1=bc_sb[:],
                    op=mybir.AluOpType.mult,
                )
                nc.sync.dma_start(out=ag_in[p][qb, hh], in_=att[:])
            # per-q-block AllGather: fires as soon as both heads' attT
            # for this q block are written
            nc.gpsimd.collective_compute(
                "AllGather", mybir.AluOpType.bypass,
                replica_groups=groups,
                ins=[ag_in[p][qb].opt()],
                outs=[ag_out[p][qb].opt()],
            )


def _phase_e(nc, tc, p, ag_out, wout, y_acc_dram, y_out, groups):
    """Half output projection for pass p's heads (4 srcs x 2 heads)."""
    with (
        tc.tile_pool(name=f"ysb{p}", bufs=3) as ysbp,
        tc.tile_pool(name=f"a2asb{p}", bufs=1) as a2ap,
        tc.tile_pool(name=f"wo{p}", bufs=4) as wop,
        tc.tile_pool(name=f"psY{p}", bufs=3, space="PSUM") as psY,
    ):
        a2a_sb = a2ap.tile([128, GROUP, 2, QB], F32R, tag="a2asb", name="a2a_sb")
        rank = nc.sync.cc_rank(groups)
        # this core's QB-wide q slice of the gathered attT lives entirely in
        # AG chunk qb == rank; select it with a dynamic outer-dim offset
        agf = ag_out[p].rearrange("qb s h d q -> (qb s h) d q")
        for s in range(GROUP):
            for hh in range(2):
                nc.sync.dma_start(
                    out=a2a_sb[:, s, hh, :],
                    in_=agf[bass.ds(rank * (GROUP * 2) + (2 * s + hh), 1)],
                )
        # wout rows for this pass's 8 heads, grouped [src, hh]: H = 4s+2p+hh
        wout_p = wout.rearrange("(s four) d c -> s four d c", four=4)[:, 2 * p:2 * p + 2]
        for ct in range(NCH):
            wt = wop.tile([128, 2, GROUP, 128], F32R, tag="wt", name="wt")
            for hh in range(2):
                nc.sync.dma_start(
                    out=wt[:, hh],
                    in_=wout_p[:, hh, :, ct * 128:(ct + 1) * 128].rearrange(
                        "s d c -> d s c"),
                )
            ps_y = psY.tile([128, QB], F32, tag="y", name="ps_y")
            for s in range(GROUP):
                for hh in range(2):
                    nc.tensor.matmul(
                        ps_y[:], wt[:, hh, s, :], a2a_sb[:, s, hh, :],
                        start=(s == 0 and hh == 0),
                        stop=(s == GROUP - 1 and hh == 1),
                    )
            y_sb = ysbp.tile([128, QB], F32, tag="ysb", name="ysb")
            if p == 0:
                nc.vector.tensor_copy(out=y_sb[:], in_=ps_y[:])
                nc.sync.dma_start(out=y_acc_dram[ct], in_=y_sb[:])
            else:
                part = ysbp.tile([128, QB], F32, tag="part", name="part")
                nc.sync.dma_start(out=part[:], in_=y_acc_dram[ct])
                nc.vector.tensor_tensor(
                    out=y_sb[:], in0=ps_y[:], in1=part[:],
                    op=mybir.AluOpType.add)
                nc.sync.dma_start(
                    out=y_out[ct * 128:(ct + 1) * 128, :], in_=y_sb[:])


def build_nc():
    nc = bacc.Bacc(None, num_devices=N_CORES)

    # ---- DRAM parameters (per-core data) ----
    xT = nc.declare_dram_parameter("xT", [C, T], F32R, isOutput=False)
    wproj = nc.declare_dram_parameter("wproj", [C, NPASS, 6, 128], F32R, isOutput=False)
    wout = nc.declare_dram_parameter("wout", [H, HD, C], F32R, isOutput=False)
    cos2 = nc.declare_dram_parameter("cos2", [128, T], F32R, isOutput=False)
    sins = nc.declare_dram_parameter("sins", [128, T], F32R, isOutput=False)
    masks = nc.declare_dram_parameter("masks", [128, 4, QB], F32R, isOutput=False)
    ones = nc.declare_dram_parameter("ones", [128, 1], F32R, isOutput=False)

    kT_out = nc.declare_dram_parameter("kT_out", [HPC, 128, T], F32R, isOutput=True)
    v_out = nc.declare_dram_parameter("v_out", [HPC, NKV, 128, HD], F32R, isOutput=True)
    y_out = nc.declare_dram_parameter("y_out", [C, QB], F32, isOutput=True)

    groups = [[0, 1, 2, 3], [4, 5, 6, 7]]

    with tile.TileContext(nc) as tc:
        with (
            tc.tile_pool(name="const", bufs=1) as constp,
            tc.tile_pool(name="dram", bufs=1, space="DRAM") as dramp,
        ):
            cos_sb = constp.tile([128, T], F32R, tag="cos", name="cos_sb")
            sin_sb = constp.tile([128, T], F32R, tag="sin", name="sin_sb")
            ones_sb = constp.tile([128, 1], F32R, tag="ones", name="ones_sb")
            nc.sync.dma_start(out=cos_sb[:], in_=cos2[:])
            nc.sync.dma_start(out=sin_sb[:], in_=sins[:])
            nc.sync.dma_start(out=ones_sb[:], in_=ones[:])

            # per (pass, q-block) AllGather buffers (one contiguous tile
            # per pass so phase E can select chunk qb==rank dynamically)
            ag_in = [dramp.tile([NQB, 2, 128, QB], F32R, name=f"ag_in{p}")
                     for p in range(NPASS)]
            ag_out = [dramp.tile([NQB, GROUP, 2, 128, QB], F32R, name=f"ag_out{p}")
                      for p in range(NPASS)]
            y_acc_dram = dramp.tile([NCH, 128, QB], F32, name="y_acc")

            xT_r = xT.rearrange("(ci p) t -> p ci t", p=128)
            wproj_r = wproj.rearrange("(ci p) s j d -> p ci s j d", p=128)

            for p in range(NPASS):
                with (
                    tc.tile_pool(name=f"qkv{p}", bufs=1) as qkvp,
                    tc.tile_pool(name=f"scr{p}", bufs=2) as scrp,
                ):
                    qkT, v_sb = _phase_b(
                        nc, tc, p, qkvp, scrp, cos_sb, sin_sb,
                        wproj_r, xT_r, kT_out, v_out)
                    if p == 1:
                        # pass-0 output projection: AllGather #0 chunks have
                        # long since landed; runs on a warm PE before the
                        # pass-1 attention stream
                        _phase_e(nc, tc, 0, ag_out, wout, y_acc_dram,
                                 y_out, groups)
                    _phase_c(nc, tc, p, qkT, v_sb, scrp, ones_sb, masks,
                             ag_in, ag_out, groups)

            _phase_e(nc, tc, 1, ag_out, wout, y_acc_dram, y_out, groups)

    nc.compile()
    return nc


def _host_prep(x, w_qkv, w_out, start_pos):
    x = np.asarray(x, dtype=np.float32)
    w_qkv = np.asarray(w_qkv, dtype=np.float32)
    w_out = np.asarray(w_out, dtype=np.float32)
    sp = int(start_pos)

    xT = [np.ascontiguousarray(x[b].T) for b in range(B)]

    # RoPE tables in [hd, T] layout (halves stacked), sin with baked sign
    inv_freq = 1.0 / (ROPE_BASE ** (np.arange(0, HD, 2, dtype=np.float32) / HD))
    pos = (sp + np.arange(T)).astype(np.float32)
    ang = np.outer(pos, inv_freq)          # [T, 64]
    cosv = np.cos(ang).T.astype(np.float32)  # [64, T]
    sinv = np.sin(ang).T.astype(np.float32)
    cos2 = np.ascontiguousarray(np.concatenate([cosv, cosv], 0))    # [128, T]
    sins = np.ascontiguousarray(np.concatenate([-sinv, sinv], 0))   # [128, T]

    # causal 0/1 masks for the 4 diagonal chunk offsets: [kv,(r),q]
    kv = np.arange(128)[:, None]
    q = np.arange(QB)[None, :]
    masks = np.stack(
        [(128 * r + kv <= q).astype(np.float32) for r in range(4)], axis=1)

    ones = np.ones((128, 1), np.float32)

    # per-core weight slices: wproj[c] = [C, pass, 6, 128]
    wq = w_qkv[:, 0 * C:1 * C].reshape(C, H, HD)
    wk = w_qkv[:, 1 * C:2 * C].reshape(C, H, HD)
    wv = w_qkv[:, 2 * C:3 * C].reshape(C, H, HD)
    wproj_cores = []
    for hg in range(GROUP):
        parts = []
        for p in range(NPASS):
            h0, h1 = 4 * hg + 2 * p, 4 * hg + 2 * p + 1
            parts.append(np.stack(
                [wq[:, h0], wq[:, h1], wk[:, h0], wk[:, h1],
                 wv[:, h0], wv[:, h1]], axis=1))   # [C, 6, 128]
        wproj_cores.append(np.ascontiguousarray(
            np.stack(parts, axis=1)))              # [C, 2, 6, 128]

    wout_r = np.ascontiguousarray(w_out.reshape(H, HD, C))

    in_maps = []
    for c in range(N_CORES):
        b, hg = c // GROUP, c % GROUP
        in_maps.append({
            "xT": xT[b],
            "wproj": wproj_cores[hg],
            "wout": wout_r,
            "cos2": cos2,
            "sins": sins,
            "masks": masks,
            "ones": ones,
        })
    return in_maps


def _assemble(results):
    y = np.empty((B, T, C), np.float32)
    k = np.empty((B, H, T, HD), np.float32)
    v = np.empty((B, H, T, HD), np.float32)
    for c in range(N_CORES):
        b, hg = c // GROUP, c % GROUP
        r = results[c]
        y[b, QB * hg:QB * (hg + 1), :] = r["y_out"].T
        for lh in range(HPC):
            hglob = 4 * hg + lh
            k[b, hglob] = r["kT_out"][lh].T
            v[b, hglob] = r["v_out"][lh].reshape(T, HD)
    return y, k, v


def run(x, w_qkv, w_out, start_pos, **spmd_kwargs):
    if "nc" not in _CACHED:
        _CACHED["nc"] = build_nc()
    nc = _CACHED["nc"]
    in_maps = _host_prep(x, w_qkv, w_out, start_pos)
    res = run_bass_kernel_spmd(nc, in_maps, list(range(N_CORES)), **spmd_kwargs)
    return _assemble(res.results), res


def kernel(x, w_qkv, w_out, start_pos):
    (y, k, v), _ = run(x, w_qkv, w_out, start_pos)
    return y, k, v


# revision 15
# speedup vs baseline: 1.2321x; 1.2183x over previous
"""Tensor-parallel causal self-attention kernel for 8 Trainium2 NeuronCores.

Problem: B=2, T=2048, C=2048, H=16 heads x hd=128, rotate-half RoPE,
causal softmax, out projection. Returns (y, k, v) with k, v post-RoPE in
[B, H, T, hd] layout.

Sharding: TP-4 over heads x DP-2 over batch. Core c handles batch c//4 and
global heads {4*(c%4) .. 4*(c%4)+3}. Each core computes QKV for its heads
(fp32r matmuls, [out_dim, T] transposed layout), RoPE, causal attention in
S^T=[kv,q] layout, then per-q-block AllGathers within each batch group of 4
cores exchange attention outputs so every core computes the full
w_out^T @ attT for its own 512-query slice of yT (selected with a
cc_rank-driven dynamic DMA offset). The output projection is split per
head-pair pass so half of it overlaps the second pass's attention.
"""

import os
import sys

sys.path.insert(0, "/opt/trn_rl_repo")

import ml_dtypes
import numpy as np

import concourse.bass as bass
import concourse.mybir as mybir
import concourse.tile as tile
from concourse import bacc
from concourse.bass_utils import run_bass_kernel_spmd

F32 = mybir.dt.float32
F32R = mybir.dt.float32r
BF16 = mybir.dt.bfloat16

N_CORES = 8
B, T, C = 2, 2048, 2048
H, HD = 16, 128
ROPE_BASE = 10000.0

GROUP = 4            # cores per batch group (TP degree)
HPC = H // GROUP     # heads per core = 4
NPASS = 2            # head-pairs per core
QB = 512             # q-block / nt-slice width
NQB = T // QB        # 4
NCH = C // 128       # 16 contraction chunks
NKV = T // 128       # 16 kv chunks
SCALE = 1.0 / float(np.sqrt(HD))

_CACHED = {}


def _phase_b(nc, tc, p, qkvp, scrp, cos_sb, sin_sb, wproj_r, xT_r, kT_out, v_out):
    """QKV projection + fused RoPE for head pair p. Returns (qkT, v_sb)."""
    qkT = qkvp.tile([128, 2, 2, T], F32R, tag="qkT", name="qkT")  # [d, hh, q/k, t]
    v_sb = qkvp.tile([128, 2, NKV, HD], F32R, tag="v", name="v_sb")
    # bf16 shadows feed the attention matmuls; the f32r versions feed the
    # k/v outputs so those keep full precision
    qk_bf = qkvp.tile([128, 2, 2, T], BF16, tag="qkbf", name="qk_bf")
    v_bf = qkvp.tile([128, 2, NKV, HD], BF16, tag="vbf", name="v_bf")

    with (
        tc.tile_pool(name=f"xw{p}", bufs=2) as xwp,
        tc.tile_pool(name=f"wc{p}", bufs=1) as wcp,
        tc.tile_pool(name=f"psB{p}", bufs=3, space="PSUM") as psB,
    ):
        wc = wcp.tile([128, NCH, 6, 128], BF16, tag="wc", name="wc")
        for ci in range(NCH):
            nc.sync.dma_start(out=wc[:, ci], in_=wproj_r[:, ci, p])

        for nt in range(NQB):
            xh = [xwp.tile([128, 8, QB], BF16, tag=f"xh{i}", name=f"xh{i}")
                  for i in range(2)]
            for i in range(2):
                nc.sync.dma_start(
                    out=xh[i][:],
                    in_=xT_r[:, 8 * i:8 * (i + 1), nt * QB:(nt + 1) * QB],
                )

            # q/k outputs: 4 targets (hh x q/k), accumulate 16 chunks
            for hh in range(2):
                for qk in range(2):
                    ps = psB.tile([128, QB], F32, tag="qk", name="ps_qk")
                    for ci in range(NCH):
                        nc.tensor.matmul(
                            ps[:],
                            wc[:, ci, 2 * qk + hh],
                            xh[ci // 8][:, ci % 8],
                            start=(ci == 0),
                            stop=(ci == NCH - 1),
                        )
                    # RoPE fused with PSUM->SBUF: out = res*cos + rot(res)*sin'
                    dst = qkT[:, hh, qk, nt * QB:(nt + 1) * QB]
                    cs = cos_sb[:, nt * QB:(nt + 1) * QB]
                    sn = sin_sb[:, nt * QB:(nt + 1) * QB]
                    rot = scrp.tile([128, QB], F32, tag="rot", name="rot")
                    tmp = scrp.tile([128, QB], F32, tag="tmp", name="tmp")
                    nc.vector.tensor_copy(out=rot[0:64, :], in_=ps[64:128, :])
                    nc.vector.tensor_copy(out=rot[64:128, :], in_=ps[0:64, :])
                    nc.vector.tensor_tensor(
                        out=dst, in0=ps[:], in1=cs, op=mybir.AluOpType.mult)
                    nc.vector.tensor_tensor(
                        out=tmp[:], in0=rot[:], in1=sn, op=mybir.AluOpType.mult)
                    nc.vector.tensor_tensor(
                        out=dst, in0=dst, in1=tmp[:], op=mybir.AluOpType.add)
                    nc.vector.tensor_copy(
                        out=qk_bf[:, hh, qk, nt * QB:(nt + 1) * QB], in_=dst)

            # v outputs: natural layout, t-tiles 4nt..4nt+3
            for tt in range(4):
                psv = psB.tile([128, 256], F32, tag="v", name="ps_v")
                for ci in range(NCH):
                    nc.tensor.matmul(
                        psv[:],
                        xh[ci // 8][:, ci % 8, tt * 128:(tt + 1) * 128],
                        wc[:, ci, 4:6].rearrange("p j d -> p (j d)"),
                        start=(ci == 0),
                        stop=(ci == NCH - 1),
                    )
                for hh in range(2):
                    nc.vector.tensor_copy(
                        out=v_sb[:, hh, 4 * nt + tt, :],
                        in_=psv[:, hh * HD:(hh + 1) * HD],
                    )
                    nc.vector.tensor_copy(
                        out=v_bf[:, hh, 4 * nt + tt, :],
                        in_=psv[:, hh * HD:(hh + 1) * HD],
                    )

        # write k, v outputs for this pass's heads
        for hh in range(2):
            lh = 2 * p + hh
            nc.sync.dma_start(out=kT_out[lh], in_=qkT[:, hh, 1, :])
            nc.sync.dma_start(
                out=v_out[lh].rearrange("c pp d -> pp c d"),
                in_=v_sb[:, hh],
            )
    return qk_bf, v_bf


def _phase_c(nc, tc, p, qkT, v_sb, scrp, ones_sb, masks, ag_in, ag_out, groups):
    """Causal attention in S^T layout + per-q-block AllGathers."""
    with (
        tc.tile_pool(name=f"pt{p}", bufs=4) as ptp,
        tc.tile_pool(name=f"at{p}", bufs=3) as atp,
        tc.tile_pool(name=f"mk{p}", bufs=1) as mkp,
        tc.tile_pool(name=f"psS{p}", bufs=3, space="PSUM") as psS,
        tc.tile_pool(name=f"psO{p}", bufs=2, space="PSUM") as psO,
        tc.tile_pool(name=f"psD{p}", bufs=2, space="PSUM") as psD,
    ):
        mask_sb = mkp.tile([128, 4, QB], BF16, tag="mask", name="mask_sb")
        nc.sync.dma_start(out=mask_sb[:], in_=masks[:])
        for qb in range(NQB):
            for hh in range(2):
                nj = 4 * qb + 4  # kv chunks for this q block
                ps_o = psO.tile([128, QB], F32, tag="o", name="ps_o")
                den = atp.tile([128, QB], F32R, tag="den", name="den")
                qrhs = qkT[:, hh, 0, qb * QB:(qb + 1) * QB]
                pts = []
                for j in range(nj):
                    ps_s = psS.tile([128, QB], F32, tag="s", name="ps_s")
                    nc.tensor.matmul(
                        ps_s[:],
                        qkT[:, hh, 1, j * 128:(j + 1) * 128],
                        qrhs,
                        start=True, stop=True,
                    )
                    pt = ptp.tile([128, QB], BF16, tag="pt", name="pt")
                    nc.scalar.activation(
                        pt[:], ps_s[:],
                        mybir.ActivationFunctionType.Exp,
                        scale=SCALE,
                    )
                    if j >= 4 * qb:  # diagonal chunk: causal mask
                        nc.vector.tensor_tensor(
                            out=pt[:], in0=pt[:],
                            in1=mask_sb[:, j - 4 * qb],
                            op=mybir.AluOpType.mult,
                        )
                    # softmax denominator accumulates on DVE (PE stays on
                    # scores/PV matmuls)
                    if j == 1:
                        nc.vector.tensor_tensor(
                            out=den[:], in0=pts[0][:], in1=pt[:],
                            op=mybir.AluOpType.add)
                    elif j > 1:
                        nc.vector.tensor_tensor(
                            out=den[:], in0=den[:], in1=pt[:],
                            op=mybir.AluOpType.add)
                    pts.append(pt)
                    # PV matmul for the PREVIOUS chunk: by the time the PE
                    # finishes s(j), exp(j-1) has drained, so the PE never
                    # stalls on the Activation engine
                    if j > 0:
                        nc.tensor.matmul(
                            ps_o[:], v_sb[:, hh, j - 1, :], pts[j - 1][:],
                            start=(j - 1 == 0), stop=False,
                        )
                nc.tensor.matmul(
                    ps_o[:], v_sb[:, hh, nj - 1, :], pts[nj - 1][:],
                    start=False, stop=True,
                )
                ps_d = psD.tile([1, QB], F32, tag="d", name="ps_d")
                nc.tensor.matmul(
                    ps_d[:], ones_sb[:], den[:], start=True, stop=True)
                recip = scrp.tile([1, QB], F32, tag="recip", name="recip")
                nc.vector.reciprocal(recip[:], ps_d[:])
                bc_sb = atp.tile([128, QB], F32, tag="bcs", name="bc_sb")
                nc.gpsimd.partition_broadcast(bc_sb[:], recip[:])
                att = atp.tile([128, QB], BF16, tag="att", name="att")
                nc.vector.tensor_tensor(
                    out=att[:], in0=ps_o[:], in1=bc_sb[:],
                    op=mybir.AluOpType.mult,
                )
                nc.sync.dma_start(out=ag_in[p][qb, hh], in_=att[:])
            # per-q-block AllGather: fires as soon as both heads' attT
            # for this q block are written
            nc.gpsimd.collective_compute(
                "AllGather", mybir.AluOpType.bypass,
                replica_groups=groups,
                ins=[ag_in[p][qb].opt()],
                outs=[ag_out[p][qb].opt()],
            )


def _phase_e(nc, tc, p, ag_out, wout, y_acc_dram, y_out, groups):
    """Half output projection for pass p's heads (4 srcs x 2 heads)."""
    with (
        tc.tile_pool(name=f"ysb{p}", bufs=3) as ysbp,
        tc.tile_pool(name=f"a2asb{p}", bufs=1) as a2ap,
        tc.tile_pool(name=f"wo{p}", bufs=4) as wop,
        tc.tile_pool(name=f"psY{p}", bufs=3, space="PSUM") as psY,
    ):
        a2a_sb = a2ap.tile([128, GROUP, 2, QB], BF16, tag="a2asb", name="a2a_sb")
        rank = nc.sync.cc_rank(groups)
        # this core's QB-wide q slice of the gathered attT lives entirely in
        # AG chunk qb == rank; select it with a dynamic outer-dim offset
        agf = ag_out[p].rearrange("qb s h d q -> (qb s h) d q")
        for s in range(GROUP):
            for hh in range(2):
                nc.sync.dma_start(
                    out=a2a_sb[:, s, hh, :],
                    in_=agf[bass.ds(rank * (GROUP * 2) + (2 * s + hh), 1)],
                )
        # wout rows for this pass's 8 heads, grouped [src, hh]: H = 4s+2p+hh
        wout_p = wout.rearrange("(s four) d c -> s four d c", four=4)[:, 2 * p:2 * p + 2]
        for ct in range(NCH):
            wt = wop.tile([128, 2, GROUP, 128], BF16, tag="wt", name="wt")
            for hh in range(2):
                nc.sync.dma_start(
                    out=wt[:, hh],
                    in_=wout_p[:, hh, :, ct * 128:(ct + 1) * 128].rearrange(
                        "s d c -> d s c"),
                )
            ps_y = psY.tile([128, QB], F32, tag="y", name="ps_y")
            for s in range(GROUP):
                for hh in range(2):
                    nc.tensor.matmul(
                        ps_y[:], wt[:, hh, s, :], a2a_sb[:, s, hh, :],
                        start=(s == 0 and hh == 0),
                        stop=(s == GROUP - 1 and hh == 1),
                    )
            y_sb = ysbp.tile([128, QB], F32, tag="ysb", name="ysb")
            if p == 0:
                nc.vector.tensor_copy(out=y_sb[:], in_=ps_y[:])
                nc.sync.dma_start(out=y_acc_dram[ct], in_=y_sb[:])
            else:
                part = ysbp.tile([128, QB], F32, tag="part", name="part")
                nc.sync.dma_start(out=part[:], in_=y_acc_dram[ct])
                nc.vector.tensor_tensor(
                    out=y_sb[:], in0=ps_y[:], in1=part[:],
                    op=mybir.AluOpType.add)
                nc.sync.dma_start(
                    out=y_out[ct * 128:(ct + 1) * 128, :], in_=y_sb[:])


def build_nc():
    nc = bacc.Bacc(None, num_devices=N_CORES)

    # ---- DRAM parameters (per-core data) ----
    xT = nc.declare_dram_parameter("xT", [C, T], BF16, isOutput=False)
    wproj = nc.declare_dram_parameter("wproj", [C, NPASS, 6, 128], BF16, isOutput=False)
    wout = nc.declare_dram_parameter("wout", [H, HD, C], BF16, isOutput=False)
    cos2 = nc.declare_dram_parameter("cos2", [128, T], F32R, isOutput=False)
    sins = nc.declare_dram_parameter("sins", [128, T], F32R, isOutput=False)
    masks = nc.declare_dram_parameter("masks", [128, 4, QB], BF16, isOutput=False)
    ones = nc.declare_dram_parameter("ones", [128, 1], F32R, isOutput=False)

    kT_out = nc.declare_dram_parameter("kT_out", [HPC, 128, T], F32R, isOutput=True)
    v_out = nc.declare_dram_parameter("v_out", [HPC, NKV, 128, HD], F32R, isOutput=True)
    y_out = nc.declare_dram_parameter("y_out", [C, QB], F32, isOutput=True)

    groups = [[0, 1, 2, 3], [4, 5, 6, 7]]

    with tile.TileContext(nc) as tc:
        with (
            tc.tile_pool(name="const", bufs=1) as constp,
            tc.tile_pool(name="dram", bufs=1, space="DRAM") as dramp,
        ):
            cos_sb = constp.tile([128, T], F32R, tag="cos", name="cos_sb")
            sin_sb = constp.tile([128, T], F32R, tag="sin", name="sin_sb")
            ones_sb = constp.tile([128, 1], F32R, tag="ones", name="ones_sb")
            nc.sync.dma_start(out=cos_sb[:], in_=cos2[:])
            nc.sync.dma_start(out=sin_sb[:], in_=sins[:])
            nc.sync.dma_start(out=ones_sb[:], in_=ones[:])

            # per (pass, q-block) AllGather buffers (one contiguous tile
            # per pass so phase E can select chunk qb==rank dynamically)
            ag_in = [dramp.tile([NQB, 2, 128, QB], BF16, name=f"ag_in{p}")
                     for p in range(NPASS)]
            ag_out = [dramp.tile([NQB, GROUP, 2, 128, QB], BF16, name=f"ag_out{p}")
                      for p in range(NPASS)]
            y_acc_dram = dramp.tile([NCH, 128, QB], F32, name="y_acc")

            xT_r = xT.rearrange("(ci p) t -> p ci t", p=128)
            wproj_r = wproj.rearrange("(ci p) s j d -> p ci s j d", p=128)

            for p in range(NPASS):
                with (
                    tc.tile_pool(name=f"qkv{p}", bufs=1) as qkvp,
                    tc.tile_pool(name=f"scr{p}", bufs=2) as scrp,
                ):
                    qk_bf, v_bf = _phase_b(
                        nc, tc, p, qkvp, scrp, cos_sb, sin_sb,
                        wproj_r, xT_r, kT_out, v_out)
                    if p == 1:
                        # pass-0 output projection: AllGather #0 chunks have
                        # long since landed; runs on a warm PE before the
                        # pass-1 attention stream
                        _phase_e(nc, tc, 0, ag_out, wout, y_acc_dram,
                                 y_out, groups)
                    _phase_c(nc, tc, p, qk_bf, v_bf, scrp, ones_sb, masks,
                             ag_in, ag_out, groups)

            _phase_e(nc, tc, 1, ag_out, wout, y_acc_dram, y_out, groups)

    nc.compile()
    return nc


def _host_prep(x, w_qkv, w_out, start_pos):
    x = np.asarray(x, dtype=np.float32)
    w_qkv = np.asarray(w_qkv, dtype=np.float32)
    w_out = np.asarray(w_out, dtype=np.float32)
    sp = int(start_pos)

    xT = [np.ascontiguousarray(x[b].T).astype(ml_dtypes.bfloat16) for b in range(B)]

    # RoPE tables in [hd, T] layout (halves stacked), sin with baked sign
    inv_freq = 1.0 / (ROPE_BASE ** (np.arange(0, HD, 2, dtype=np.float32) / HD))
    pos = (sp + np.arange(T)).astype(np.float32)
    ang = np.outer(pos, inv_freq)          # [T, 64]
    cosv = np.cos(ang).T.astype(np.float32)  # [64, T]
    sinv = np.sin(ang).T.astype(np.float32)
    cos2 = np.ascontiguousarray(np.concatenate([cosv, cosv], 0))    # [128, T]
    sins = np.ascontiguousarray(np.concatenate([-sinv, sinv], 0))   # [128, T]

    # causal 0/1 masks for the 4 diagonal chunk offsets: [kv,(r),q]
    kv = np.arange(128)[:, None]
    q = np.arange(QB)[None, :]
    masks = np.stack(
        [(128 * r + kv <= q).astype(ml_dtypes.bfloat16) for r in range(4)], axis=1)

    ones = np.ones((128, 1), np.float32)

    # per-core weight slices: wproj[c] = [C, pass, 6, 128]
    wq = w_qkv[:, 0 * C:1 * C].reshape(C, H, HD)
    wk = w_qkv[:, 1 * C:2 * C].reshape(C, H, HD)
    wv = w_qkv[:, 2 * C:3 * C].reshape(C, H, HD)
    wproj_cores = []
    for hg in range(GROUP):
        parts = []
        for p in range(NPASS):
            h0, h1 = 4 * hg + 2 * p, 4 * hg + 2 * p + 1
            parts.append(np.stack(
                [wq[:, h0], wq[:, h1], wk[:, h0], wk[:, h1],
                 wv[:, h0], wv[:, h1]], axis=1))   # [C, 6, 128]
        wproj_cores.append(np.ascontiguousarray(
            np.stack(parts, axis=1)).astype(ml_dtypes.bfloat16))  # [C, 2, 6, 128]

    wout_r = np.ascontiguousarray(w_out.reshape(H, HD, C)).astype(ml_dtypes.bfloat16)

    in_maps = []
    for c in range(N_CORES):
        b, hg = c // GROUP, c % GROUP
        in_maps.append({
            "xT": xT[b],
            "wproj": wproj_cores[hg],
            "wout": wout_r,
            "cos2": cos2,
            "sins": sins,
            "masks": masks,
            "ones": ones,
        })
    return in_maps


def _assemble(results):
    y = np.empty((B, T, C), np.float32)
    k = np.empty((B, H, T, HD), np.float32)
    v = np.empty((B, H, T, HD), np.float32)
    for c in range(N_CORES):
        b, hg = c // GROUP, c % GROUP
        r = results[c]
        y[b, QB * hg:QB * (hg + 1), :] = r["y_out"].T
        for lh in range(HPC):
            hglob = 4 * hg + lh
            k[b, hglob] = r["kT_out"][lh].T
            v[b, hglob] = r["v_out"][lh].reshape(T, HD)
    return y, k, v


def run(x, w_qkv, w_out, start_pos, **spmd_kwargs):
    if "nc" not in _CACHED:
        _CACHED["nc"] = build_nc()
    nc = _CACHED["nc"]
    in_maps = _host_prep(x, w_qkv, w_out, start_pos)
    res = run_bass_kernel_spmd(nc, in_maps, list(range(N_CORES)), **spmd_kwargs)
    return _assemble(res.results), res


def kernel(x, w_qkv, w_out, start_pos):
    (y, k, v), _ = run(x, w_qkv, w_out, start_pos)
    return y, k, v


# revision 16
# speedup vs baseline: 1.2757x; 1.0354x over previous
"""Tensor-parallel causal self-attention kernel for 8 Trainium2 NeuronCores.

Problem: B=2, T=2048, C=2048, H=16 heads x hd=128, rotate-half RoPE,
causal softmax, out projection. Returns (y, k, v) with k, v post-RoPE in
[B, H, T, hd] layout.

Sharding: TP-4 over heads x DP-2 over batch. Core c handles batch c//4 and
global heads {4*(c%4) .. 4*(c%4)+3}. Each core computes QKV for its heads
(fp32r matmuls, [out_dim, T] transposed layout), RoPE, causal attention in
S^T=[kv,q] layout, then per-q-block AllGathers within each batch group of 4
cores exchange attention outputs so every core computes the full
w_out^T @ attT for its own 512-query slice of yT (selected with a
cc_rank-driven dynamic DMA offset). The output projection is split per
head-pair pass so half of it overlaps the second pass's attention.
"""

import os
import sys

sys.path.insert(0, "/opt/trn_rl_repo")

import ml_dtypes
import numpy as np

import concourse.bass as bass
import concourse.mybir as mybir
import concourse.tile as tile
from concourse import bacc
from concourse.bass_utils import run_bass_kernel_spmd

F32 = mybir.dt.float32
F32R = mybir.dt.float32r
BF16 = mybir.dt.bfloat16

N_CORES = 8
B, T, C = 2, 2048, 2048
H, HD = 16, 128
ROPE_BASE = 10000.0

GROUP = 4            # cores per batch group (TP degree)
HPC = H // GROUP     # heads per core = 4
NPASS = 2            # head-pairs per core
QB = 512             # q-block / nt-slice width
NQB = T // QB        # 4
NCH = C // 128       # 16 contraction chunks
NKV = T // 128       # 16 kv chunks
SCALE = 1.0 / float(np.sqrt(HD))

_CACHED = {}


def _phase_b(nc, tc, p, qkvp, scrp, v_sb, v_bf, cos_sb, sin_sb, wproj_r,
             xT_r, kT_out, v_out):
    """QKV projection + fused RoPE for head pair p. V for ALL 4 heads is
    projected in pass 0 (N=512 moving operand instead of 2x N=256)."""
    qkT = qkvp.tile([128, 2, 2, T], F32R, tag="qkT", name="qkT")  # [d, hh, q/k, t]
    qk_bf = qkvp.tile([128, 2, 2, T], BF16, tag="qkbf", name="qk_bf")

    nslot = 8 if p == 0 else 4
    soff = 0 if p == 0 else 8
    with (
        tc.tile_pool(name=f"xw{p}", bufs=2) as xwp,
        tc.tile_pool(name=f"wc{p}", bufs=1) as wcp,
        tc.tile_pool(name=f"psB{p}", bufs=3, space="PSUM") as psB,
    ):
        wc = wcp.tile([128, NCH, nslot, 128], BF16, tag="wc", name="wc")
        for ci in range(NCH):
            nc.sync.dma_start(out=wc[:, ci], in_=wproj_r[:, ci, soff:soff + nslot])

        for nt in range(NQB):
            xh = [xwp.tile([128, 8, QB], BF16, tag=f"xh{i}", name=f"xh{i}")
                  for i in range(2)]
            for i in range(2):
                nc.sync.dma_start(
                    out=xh[i][:],
                    in_=xT_r[:, 8 * i:8 * (i + 1), nt * QB:(nt + 1) * QB],
                )

            # q/k outputs: 4 targets (hh x q/k), accumulate 16 chunks
            for hh in range(2):
                for qk in range(2):
                    ps = psB.tile([128, QB], F32, tag="qk", name="ps_qk")
                    for ci in range(NCH):
                        nc.tensor.matmul(
                            ps[:],
                            wc[:, ci, 2 * qk + hh],
                            xh[ci // 8][:, ci % 8],
                            start=(ci == 0),
                            stop=(ci == NCH - 1),
                        )
                    # RoPE fused with PSUM->SBUF: out = res*cos + rot(res)*sin'
                    dst = qkT[:, hh, qk, nt * QB:(nt + 1) * QB]
                    cs = cos_sb[:, nt * QB:(nt + 1) * QB]
                    sn = sin_sb[:, nt * QB:(nt + 1) * QB]
                    rot = scrp.tile([128, QB], F32, tag="rot", name="rot")
                    tmp = scrp.tile([128, QB], F32, tag="tmp", name="tmp")
                    nc.vector.tensor_copy(out=rot[0:64, :], in_=ps[64:128, :])
                    nc.vector.tensor_copy(out=rot[64:128, :], in_=ps[0:64, :])
                    nc.vector.tensor_tensor(
                        out=dst, in0=ps[:], in1=cs, op=mybir.AluOpType.mult)
                    nc.vector.tensor_tensor(
                        out=tmp[:], in0=rot[:], in1=sn, op=mybir.AluOpType.mult)
                    nc.vector.tensor_tensor(
                        out=dst, in0=dst, in1=tmp[:], op=mybir.AluOpType.add)
                    nc.vector.tensor_copy(
                        out=qk_bf[:, hh, qk, nt * QB:(nt + 1) * QB], in_=dst)

            if p == 0:
                # v for all 4 heads: t-tiles 4nt..4nt+3, N=512 moving
                for tt in range(4):
                    psv = psB.tile([128, QB], F32, tag="v", name="ps_v")
                    for ci in range(NCH):
                        nc.tensor.matmul(
                            psv[:],
                            xh[ci // 8][:, ci % 8, tt * 128:(tt + 1) * 128],
                            wc[:, ci, 4:8].rearrange("pp j d -> pp (j d)"),
                            start=(ci == 0),
                            stop=(ci == NCH - 1),
                        )
                    for lh in range(4):
                        nc.vector.tensor_copy(
                            out=v_sb[:, lh, 4 * nt + tt, :],
                            in_=psv[:, lh * HD:(lh + 1) * HD],
                        )
                        nc.vector.tensor_copy(
                            out=v_bf[:, lh, 4 * nt + tt, :],
                            in_=psv[:, lh * HD:(lh + 1) * HD],
                        )

        # write k (this pass) and v (pass 0 wrote all heads) outputs
        for hh in range(2):
            lh = 2 * p + hh
            nc.sync.dma_start(out=kT_out[lh], in_=qkT[:, hh, 1, :])
        if p == 0:
            for lh in range(4):
                nc.sync.dma_start(
                    out=v_out[lh].rearrange("c pp d -> pp c d"),
                    in_=v_sb[:, lh],
                )
    return qk_bf


def _phase_c(nc, tc, p, qk_bf, v_bf, scrp, ones_sb, masks, ag_in, ag_out,
             groups):
    """Causal attention in S^T layout + per-q-block AllGathers.

    The per-round finalize (ones-matmul denominator, reciprocal, broadcast,
    normalize, DMA) is deferred into the NEXT round's score stream so the PE
    never waits on the DVE denominator chain."""
    with (
        tc.tile_pool(name=f"pt{p}", bufs=6) as ptp,
        tc.tile_pool(name=f"at{p}", bufs=3) as atp,
        tc.tile_pool(name=f"mk{p}", bufs=1) as mkp,
        tc.tile_pool(name=f"psS{p}", bufs=4, space="PSUM") as psS,
        tc.tile_pool(name=f"psO{p}", bufs=2, space="PSUM") as psO,
        tc.tile_pool(name=f"psD{p}", bufs=2, space="PSUM") as psD,
    ):
        mask_sb = mkp.tile([128, 4, QB], BF16, tag="mask", name="mask_sb")
        nc.sync.dma_start(out=mask_sb[:], in_=masks[:])

        pending = []

        def flush_pending():
            if not pending:
                return
            hh_, qb_, ps_o_, den_ = pending.pop()
            ps_d = psD.tile([1, QB], F32, tag="d", name="ps_d")
            nc.tensor.matmul(
                ps_d[:], ones_sb[:], den_[:], start=True, stop=True)
            recip = scrp.tile([1, QB], F32, tag="recip", name="recip")
            nc.vector.reciprocal(recip[:], ps_d[:])
            bc_sb = atp.tile([128, QB], F32, tag="bcs", name="bc_sb")
            nc.gpsimd.partition_broadcast(bc_sb[:], recip[:])
            att = atp.tile([128, QB], BF16, tag="att", name="att")
            nc.vector.tensor_tensor(
                out=att[:], in0=ps_o_[:], in1=bc_sb[:],
                op=mybir.AluOpType.mult,
            )
            nc.sync.dma_start(out=ag_in[p][qb_, hh_], in_=att[:])
            if hh_ == 1:
                # both heads of this q block written: gather it
                nc.gpsimd.collective_compute(
                    "AllGather", mybir.AluOpType.bypass,
                    replica_groups=groups,
                    ins=[ag_in[p][qb_].opt()],
                    outs=[ag_out[p][qb_].opt()],
                )

        for qb in range(NQB):
            for hh in range(2):
                nj = 4 * qb + 4  # kv chunks for this q block
                ps_o = psO.tile([128, QB], F32, tag="o", name="ps_o")
                den = atp.tile([128, QB], F32R, tag="den", name="den")
                qrhs = qk_bf[:, hh, 0, qb * QB:(qb + 1) * QB]
                pts = []
                for j in range(nj):
                    ps_s = psS.tile([128, QB], F32, tag="s", name="ps_s")
                    nc.tensor.matmul(
                        ps_s[:],
                        qk_bf[:, hh, 1, j * 128:(j + 1) * 128],
                        qrhs,
                        start=True, stop=True,
                    )
                    if j == 1:
                        flush_pending()  # previous round's finalize
                    pt = ptp.tile([128, QB], BF16, tag="pt", name="pt")
                    nc.scalar.activation(
                        pt[:], ps_s[:],
                        mybir.ActivationFunctionType.Exp,
                        scale=SCALE,
                    )
                    if j >= 4 * qb:  # diagonal chunk: causal mask
                        nc.vector.tensor_tensor(
                            out=pt[:], in0=pt[:],
                            in1=mask_sb[:, j - 4 * qb],
                            op=mybir.AluOpType.mult,
                        )
                    # softmax denominator accumulates on DVE (PE stays on
                    # scores/PV matmuls)
                    if j == 1:
                        nc.vector.tensor_tensor(
                            out=den[:], in0=pts[0][:], in1=pt[:],
                            op=mybir.AluOpType.add)
                    elif j > 1:
                        nc.vector.tensor_tensor(
                            out=den[:], in0=den[:], in1=pt[:],
                            op=mybir.AluOpType.add)
                    pts.append(pt)
                    # PV matmul for the PREVIOUS chunk: by the time the PE
                    # finishes s(j), exp(j-1) has drained, so the PE never
                    # stalls on the Activation engine
                    if j > 0:
                        nc.tensor.matmul(
                            ps_o[:], v_bf[:, 2 * p + hh, j - 1, :],
                            pts[j - 1][:],
                            start=(j - 1 == 0), stop=False,
                        )
                nc.tensor.matmul(
                    ps_o[:], v_bf[:, 2 * p + hh, nj - 1, :], pts[nj - 1][:],
                    start=False, stop=True,
                )
                pending.append((hh, qb, ps_o, den))
        flush_pending()


def _phase_e(nc, tc, p, ag_out, wout, y_acc_dram, y_out, groups):
    """Half output projection for pass p's heads (4 srcs x 2 heads)."""
    with (
        tc.tile_pool(name=f"ysb{p}", bufs=3) as ysbp,
        tc.tile_pool(name=f"a2asb{p}", bufs=1) as a2ap,
        tc.tile_pool(name=f"wo{p}", bufs=1) as wop,
        tc.tile_pool(name=f"psY{p}", bufs=3, space="PSUM") as psY,
    ):
        # weight tiles have no dependence on the AllGather: issue all their
        # DMAs first so they stream in while the collective completes
        wout_p = wout.rearrange("(s four) d c -> s four d c", four=4)[:, 2 * p:2 * p + 2]
        wts = []
        for ct in range(NCH):
            wt = wop.tile([128, 2, GROUP, 128], BF16, tag=f"wt{ct}",
                          name=f"wt{ct}")
            for hh in range(2):
                nc.sync.dma_start(
                    out=wt[:, hh],
                    in_=wout_p[:, hh, :, ct * 128:(ct + 1) * 128].rearrange(
                        "s d c -> d s c"),
                )
            wts.append(wt)

        a2a_sb = a2ap.tile([128, GROUP, 2, QB], BF16, tag="a2asb", name="a2a_sb")
        rank = nc.sync.cc_rank(groups)
        # this core's QB-wide q slice of the gathered attT lives entirely in
        # AG chunk qb == rank; select it with a dynamic outer-dim offset
        agf = ag_out[p].rearrange("qb s h d q -> (qb s h) d q")
        for s in range(GROUP):
            for hh in range(2):
                nc.sync.dma_start(
                    out=a2a_sb[:, s, hh, :],
                    in_=agf[bass.ds(rank * (GROUP * 2) + (2 * s + hh), 1)],
                )
        for ct in range(NCH):
            ps_y = psY.tile([128, QB], F32, tag="y", name="ps_y")
            for s in range(GROUP):
                for hh in range(2):
                    nc.tensor.matmul(
                        ps_y[:], wts[ct][:, hh, s, :], a2a_sb[:, s, hh, :],
                        start=(s == 0 and hh == 0),
                        stop=(s == GROUP - 1 and hh == 1),
                    )
            y_sb = ysbp.tile([128, QB], F32, tag="ysb", name="ysb")
            if p == 0:
                nc.vector.tensor_copy(out=y_sb[:], in_=ps_y[:])
                nc.sync.dma_start(out=y_acc_dram[ct], in_=y_sb[:])
            else:
                part = ysbp.tile([128, QB], F32, tag="part", name="part")
                nc.sync.dma_start(out=part[:], in_=y_acc_dram[ct])
                nc.vector.tensor_tensor(
                    out=y_sb[:], in0=ps_y[:], in1=part[:],
                    op=mybir.AluOpType.add)
                nc.sync.dma_start(
                    out=y_out[ct * 128:(ct + 1) * 128, :], in_=y_sb[:])


def build_nc():
    nc = bacc.Bacc(None, num_devices=N_CORES)

    # ---- DRAM parameters (per-core data) ----
    xT = nc.declare_dram_parameter("xT", [C, T], BF16, isOutput=False)
    wproj = nc.declare_dram_parameter("wproj", [C, 12, 128], BF16, isOutput=False)
    wout = nc.declare_dram_parameter("wout", [H, HD, C], BF16, isOutput=False)
    cos2 = nc.declare_dram_parameter("cos2", [128, T], F32R, isOutput=False)
    sins = nc.declare_dram_parameter("sins", [128, T], F32R, isOutput=False)
    masks = nc.declare_dram_parameter("masks", [128, 4, QB], BF16, isOutput=False)
    ones = nc.declare_dram_parameter("ones", [128, 1], F32R, isOutput=False)

    kT_out = nc.declare_dram_parameter("kT_out", [HPC, 128, T], F32R, isOutput=True)
    v_out = nc.declare_dram_parameter("v_out", [HPC, NKV, 128, HD], F32R, isOutput=True)
    y_out = nc.declare_dram_parameter("y_out", [C, QB], F32, isOutput=True)

    groups = [[0, 1, 2, 3], [4, 5, 6, 7]]

    with tile.TileContext(nc) as tc:
        with (
            tc.tile_pool(name="const", bufs=1) as constp,
            tc.tile_pool(name="dram", bufs=1, space="DRAM") as dramp,
        ):
            cos_sb = constp.tile([128, T], F32R, tag="cos", name="cos_sb")
            sin_sb = constp.tile([128, T], F32R, tag="sin", name="sin_sb")
            ones_sb = constp.tile([128, 1], F32R, tag="ones", name="ones_sb")
            # v for all 4 heads lives at outer scope: projected in pass 0,
            # consumed by both attention passes
            v_sb = constp.tile([128, HPC, NKV, HD], F32R, tag="vall", name="v_sb")
            v_bf = constp.tile([128, HPC, NKV, HD], BF16, tag="vallbf", name="v_bf")

            # per (pass, q-block) AllGather buffers (one contiguous tile
            # per pass so phase E can select chunk qb==rank dynamically)
            ag_in = [dramp.tile([NQB, 2, 128, QB], BF16, name=f"ag_in{p}")
                     for p in range(NPASS)]
            ag_out = [dramp.tile([NQB, GROUP, 2, 128, QB], BF16, name=f"ag_out{p}")
                      for p in range(NPASS)]
            y_acc_dram = dramp.tile([NCH, 128, QB], F32, name="y_acc")

            xT_r = xT.rearrange("(ci p) t -> p ci t", p=128)
            wproj_r = wproj.rearrange("(ci p) j d -> p ci j d", p=128)
            # constants are needed ~40us in; issue their DMAs after the
            # first-pass weight/x DMAs so the first matmuls start sooner
            nc.sync.dma_start(out=cos_sb[:], in_=cos2[:])
            nc.sync.dma_start(out=sin_sb[:], in_=sins[:])
            nc.sync.dma_start(out=ones_sb[:], in_=ones[:])

            for p in range(NPASS):
                with (
                    tc.tile_pool(name=f"qkv{p}", bufs=1) as qkvp,
                    tc.tile_pool(name=f"scr{p}", bufs=2) as scrp,
                ):
                    qk_bf = _phase_b(
                        nc, tc, p, qkvp, scrp, v_sb, v_bf, cos_sb, sin_sb,
                        wproj_r, xT_r, kT_out, v_out)
                    if p == 1:
                        # pass-0 output projection: AllGather #0 chunks have
                        # long since landed; runs on a warm PE before the
                        # pass-1 attention stream
                        _phase_e(nc, tc, 0, ag_out, wout, y_acc_dram,
                                 y_out, groups)
                    _phase_c(nc, tc, p, qk_bf, v_bf, scrp, ones_sb, masks,
                             ag_in, ag_out, groups)

            _phase_e(nc, tc, 1, ag_out, wout, y_acc_dram, y_out, groups)

    nc.compile()
    return nc


def _host_prep(x, w_qkv, w_out, start_pos):
    x = np.asarray(x, dtype=np.float32)
    w_qkv = np.asarray(w_qkv, dtype=np.float32)
    w_out = np.asarray(w_out, dtype=np.float32)
    sp = int(start_pos)

    xT = [np.ascontiguousarray(x[b].T).astype(ml_dtypes.bfloat16) for b in range(B)]

    # RoPE tables in [hd, T] layout (halves stacked), sin with baked sign
    inv_freq = 1.0 / (ROPE_BASE ** (np.arange(0, HD, 2, dtype=np.float32) / HD))
    pos = (sp + np.arange(T)).astype(np.float32)
    ang = np.outer(pos, inv_freq)          # [T, 64]
    cosv = np.cos(ang).T.astype(np.float32)  # [64, T]
    sinv = np.sin(ang).T.astype(np.float32)
    cos2 = np.ascontiguousarray(np.concatenate([cosv, cosv], 0))    # [128, T]
    sins = np.ascontiguousarray(np.concatenate([-sinv, sinv], 0))   # [128, T]

    # causal 0/1 masks for the 4 diagonal chunk offsets: [kv,(r),q]
    kv = np.arange(128)[:, None]
    q = np.arange(QB)[None, :]
    masks = np.stack(
        [(128 * r + kv <= q).astype(ml_dtypes.bfloat16) for r in range(4)], axis=1)

    ones = np.ones((128, 1), np.float32)

    # per-core weight slices: wproj[c] = [C, pass, 6, 128]
    wq = w_qkv[:, 0 * C:1 * C].reshape(C, H, HD)
    wk = w_qkv[:, 1 * C:2 * C].reshape(C, H, HD)
    wv = w_qkv[:, 2 * C:3 * C].reshape(C, H, HD)
    wproj_cores = []
    for hg in range(GROUP):
        hs = [4 * hg + j for j in range(4)]
        slots = [wq[:, hs[0]], wq[:, hs[1]], wk[:, hs[0]], wk[:, hs[1]],
                 wv[:, hs[0]], wv[:, hs[1]], wv[:, hs[2]], wv[:, hs[3]],
                 wq[:, hs[2]], wq[:, hs[3]], wk[:, hs[2]], wk[:, hs[3]]]
        wproj_cores.append(np.ascontiguousarray(
            np.stack(slots, axis=1)).astype(ml_dtypes.bfloat16))  # [C, 12, 128]

    wout_r = np.ascontiguousarray(w_out.reshape(H, HD, C)).astype(ml_dtypes.bfloat16)

    in_maps = []
    for c in range(N_CORES):
        b, hg = c // GROUP, c % GROUP
        in_maps.append({
            "xT": xT[b],
            "wproj": wproj_cores[hg],
            "wout": wout_r,
            "cos2": cos2,
            "sins": sins,
            "masks": masks,
            "ones": ones,
        })
    return in_maps


def _assemble(results):
    y = np.empty((B, T, C), np.float32)
    k = np.empty((B, H, T, HD), np.float32)
    v = np.empty((B, H, T, HD), np.float32)
    for c in range(N_CORES):
        b, hg = c // GROUP, c % GROUP
        r = results[c]
        y[b, QB * hg:QB * (hg + 1), :] = r["y_out"].T
        for lh in range(HPC):
            hglob = 4 * hg + lh
            k[b, hglob] = r["kT_out"][lh].T
            v[b, hglob] = r["v_out"][lh].reshape(T, HD)
    return y, k, v


def run(x, w_qkv, w_out, start_pos, **spmd_kwargs):
    if "nc" not in _CACHED:
        _CACHED["nc"] = build_nc()
    nc = _CACHED["nc"]
    in_maps = _host_prep(x, w_qkv, w_out, start_pos)
    res = run_bass_kernel_spmd(nc, in_maps, list(range(N_CORES)), **spmd_kwargs)
    return _assemble(res.results), res


def kernel(x, w_qkv, w_out, start_pos):
    (y, k, v), _ = run(x, w_qkv, w_out, start_pos)
    return y, k, v


# revision 18
# speedup vs baseline: 1.3061x; 1.0238x over previous
"""Tensor-parallel causal self-attention kernel for 8 Trainium2 NeuronCores.

Problem: B=2, T=2048, C=2048, H=16 heads x hd=128, rotate-half RoPE,
causal softmax, out projection. Returns (y, k, v) with k, v post-RoPE in
[B, H, T, hd] layout.

Sharding: TP-4 over heads x DP-2 over batch. Core c handles batch c//4 and
global heads {4*(c%4) .. 4*(c%4)+3}. Each core computes QKV for its heads
(fp32r matmuls, [out_dim, T] transposed layout), RoPE, causal attention in
S^T=[kv,q] layout, then per-q-block AllGathers within each batch group of 4
cores exchange attention outputs so every core computes the full
w_out^T @ attT for its own 512-query slice of yT (selected with a
cc_rank-driven dynamic DMA offset). The output projection is split per
head-pair pass so half of it overlaps the second pass's attention.
"""

import os
import sys

sys.path.insert(0, "/opt/trn_rl_repo")

import ml_dtypes
import numpy as np

import concourse.bass as bass
import concourse.mybir as mybir
import concourse.tile as tile
from concourse import bacc
from concourse.bass_utils import run_bass_kernel_spmd

F32 = mybir.dt.float32
F32R = mybir.dt.float32r
BF16 = mybir.dt.bfloat16

N_CORES = 8
B, T, C = 2, 2048, 2048
H, HD = 16, 128
ROPE_BASE = 10000.0

GROUP = 4            # cores per batch group (TP degree)
HPC = H // GROUP     # heads per core = 4
NPASS = 2            # head-pairs per core
QB = 512             # q-block / nt-slice width
NQB = T // QB        # 4
NCH = C // 128       # 16 contraction chunks
NKV = T // 128       # 16 kv chunks
SCALE = 1.0 / float(np.sqrt(HD))

_CACHED = {}


def _phase_b(nc, tc, p, qkvp, scrp, v_bf, cos_sb, sin_sb, wproj_r,
             xT_r, kT_out, v_out):
    """QKV projection + fused RoPE for head pair p. V for ALL 4 heads is
    projected in pass 0 (N=512 moving operand instead of 2x N=256)."""
    qkT = qkvp.tile([128, 2, 2, T], F32R, tag="qkT", name="qkT")  # [d, hh, q/k, t]
    qk_bf = qkvp.tile([128, 2, 2, T], BF16, tag="qkbf", name="qk_bf")

    nslot = 8 if p == 0 else 4
    soff = 0 if p == 0 else 8
    with (
        tc.tile_pool(name=f"xw{p}", bufs=2) as xwp,
        tc.tile_pool(name=f"wc{p}", bufs=1) as wcp,
        tc.tile_pool(name=f"psB{p}", bufs=3, space="PSUM") as psB,
    ):
        if p == 0:
            v_sb = wcp.tile([128, HPC, NKV, HD], F32R, tag="vall", name="v_sb")
        xhs = {}
        for nt in range(NQB):
            xhs[nt] = [xwp.tile([128, 8, QB], BF16, tag=f"xh{i}", name=f"xh{i}")
                       for i in range(2)]
        # first x slice before the weights: the first matmul needs both, and
        # the weight DMA is bigger
        for i in range(2):
            nc.sync.dma_start(
                out=xhs[0][i][:],
                in_=xT_r[:, 8 * i:8 * (i + 1), 0:QB],
            )
        wc = wcp.tile([128, NCH, nslot, 128], BF16, tag="wc", name="wc")
        for cg in range(4):
            nc.sync.dma_start(out=wc[:, 4 * cg:4 * (cg + 1)],
                              in_=wproj_r[:, 4 * cg:4 * (cg + 1), soff:soff + nslot])

        for nt in range(NQB):
            xh = xhs[nt]
            if nt > 0:
                for i in range(2):
                    nc.sync.dma_start(
                        out=xh[i][:],
                        in_=xT_r[:, 8 * i:8 * (i + 1), nt * QB:(nt + 1) * QB],
                    )

            # q/k outputs: 4 targets (hh x q/k), accumulate 16 chunks
            for hh in range(2):
                for qk in range(2):
                    ps = psB.tile([128, QB], F32, tag="qk", name="ps_qk")
                    for ci in range(NCH):
                        nc.tensor.matmul(
                            ps[:],
                            wc[:, ci, 2 * qk + hh],
                            xh[ci // 8][:, ci % 8],
                            start=(ci == 0),
                            stop=(ci == NCH - 1),
                        )
                    # RoPE fused with PSUM->SBUF: out = res*cos + rot(res)*sin'
                    dst = qkT[:, hh, qk, nt * QB:(nt + 1) * QB]
                    cs = cos_sb[:, nt * QB:(nt + 1) * QB]
                    sn = sin_sb[:, nt * QB:(nt + 1) * QB]
                    rot = scrp.tile([128, QB], F32, tag="rot", name="rot")
                    tmp = scrp.tile([128, QB], F32, tag="tmp", name="tmp")
                    nc.vector.tensor_copy(out=rot[0:64, :], in_=ps[64:128, :])
                    nc.vector.tensor_copy(out=rot[64:128, :], in_=ps[0:64, :])
                    nc.vector.tensor_tensor(
                        out=dst, in0=ps[:], in1=cs, op=mybir.AluOpType.mult)
                    nc.vector.tensor_tensor(
                        out=tmp[:], in0=rot[:], in1=sn, op=mybir.AluOpType.mult)
                    nc.vector.tensor_tensor(
                        out=dst, in0=dst, in1=tmp[:], op=mybir.AluOpType.add)
                    nc.vector.tensor_copy(
                        out=qk_bf[:, hh, qk, nt * QB:(nt + 1) * QB], in_=dst)

            if p == 0:
                # v for all 4 heads: t-tiles 4nt..4nt+3, N=512 moving
                for tt in range(4):
                    psv = psB.tile([128, QB], F32, tag="v", name="ps_v")
                    for ci in range(NCH):
                        nc.tensor.matmul(
                            psv[:],
                            xh[ci // 8][:, ci % 8, tt * 128:(tt + 1) * 128],
                            wc[:, ci, 4:8].rearrange("pp j d -> pp (j d)"),
                            start=(ci == 0),
                            stop=(ci == NCH - 1),
                        )
                    for lh in range(4):
                        nc.vector.tensor_copy(
                            out=v_sb[:, lh, 4 * nt + tt, :],
                            in_=psv[:, lh * HD:(lh + 1) * HD],
                        )
                        nc.vector.tensor_copy(
                            out=v_bf[:, lh, 4 * nt + tt, :],
                            in_=psv[:, lh * HD:(lh + 1) * HD],
                        )

        # write k (this pass) and v (pass 0 wrote all heads) outputs
        for hh in range(2):
            lh = 2 * p + hh
            nc.sync.dma_start(out=kT_out[lh], in_=qkT[:, hh, 1, :])
        if p == 0:
            for lh in range(4):
                nc.sync.dma_start(
                    out=v_out[lh].rearrange("c pp d -> pp c d"),
                    in_=v_sb[:, lh],
                )
    return qk_bf


def _phase_c(nc, tc, p, qk_bf, v_bf, scrp, ones_sb, masks, ag_in, ag_out,
             groups):
    """Causal attention in S^T layout + per-q-block AllGathers.

    The per-round finalize (ones-matmul denominator, reciprocal, broadcast,
    normalize, DMA) is deferred into the NEXT round's score stream so the PE
    never waits on the DVE denominator chain."""
    with (
        tc.tile_pool(name=f"pt{p}", bufs=6) as ptp,
        tc.tile_pool(name=f"at{p}", bufs=3) as atp,
        tc.tile_pool(name=f"mk{p}", bufs=1) as mkp,
        tc.tile_pool(name=f"psS{p}", bufs=4, space="PSUM") as psS,
        tc.tile_pool(name=f"psO{p}", bufs=2, space="PSUM") as psO,
        tc.tile_pool(name=f"psD{p}", bufs=2, space="PSUM") as psD,
    ):
        mask_sb = mkp.tile([128, 4, QB], BF16, tag="mask", name="mask_sb")
        nc.sync.dma_start(out=mask_sb[:], in_=masks[:])

        pending = []

        def flush_pending():
            if not pending:
                return
            hh_, qb_, ps_o_, den_ = pending.pop()
            ps_d = psD.tile([1, QB], F32, tag="d", name="ps_d")
            nc.tensor.matmul(
                ps_d[:], ones_sb[:], den_[:], start=True, stop=True)
            recip = scrp.tile([1, QB], F32, tag="recip", name="recip")
            nc.vector.reciprocal(recip[:], ps_d[:])
            bc_sb = atp.tile([128, QB], F32, tag="bcs", name="bc_sb")
            nc.gpsimd.partition_broadcast(bc_sb[:], recip[:])
            att = atp.tile([128, QB], BF16, tag="att", name="att")
            nc.vector.tensor_tensor(
                out=att[:], in0=ps_o_[:], in1=bc_sb[:],
                op=mybir.AluOpType.mult,
            )
            nc.sync.dma_start(out=ag_in[p][qb_, hh_], in_=att[:])
            if hh_ == 1:
                # both heads of this q block written: gather it
                nc.gpsimd.collective_compute(
                    "AllGather", mybir.AluOpType.bypass,
                    replica_groups=groups,
                    ins=[ag_in[p][qb_].opt()],
                    outs=[ag_out[p][qb_].opt()],
                )

        for qb in range(NQB):
            for hh in range(2):
                nj = 4 * qb + 4  # kv chunks for this q block
                ps_o = psO.tile([128, QB], F32, tag="o", name="ps_o")
                den = atp.tile([128, QB], F32R, tag="den", name="den")
                qrhs = qk_bf[:, hh, 0, qb * QB:(qb + 1) * QB]
                pts = []
                for j in range(nj):
                    ps_s = psS.tile([128, QB], F32, tag="s", name="ps_s")
                    nc.tensor.matmul(
                        ps_s[:],
                        qk_bf[:, hh, 1, j * 128:(j + 1) * 128],
                        qrhs,
                        start=True, stop=True,
                    )
                    if j == 1:
                        flush_pending()  # previous round's finalize
                    pt = ptp.tile([128, QB], BF16, tag="pt", name="pt")
                    nc.scalar.activation(
                        pt[:], ps_s[:],
                        mybir.ActivationFunctionType.Exp,
                        scale=SCALE,
                    )
                    if j >= 4 * qb:  # diagonal chunk: causal mask
                        nc.vector.tensor_tensor(
                            out=pt[:], in0=pt[:],
                            in1=mask_sb[:, j - 4 * qb],
                            op=mybir.AluOpType.mult,
                        )
                    # softmax denominator accumulates on DVE (PE stays on
                    # scores/PV matmuls)
                    if j == 1:
                        nc.vector.tensor_tensor(
                            out=den[:], in0=pts[0][:], in1=pt[:],
                            op=mybir.AluOpType.add)
                    elif j > 1:
                        nc.vector.tensor_tensor(
                            out=den[:], in0=den[:], in1=pt[:],
                            op=mybir.AluOpType.add)
                    pts.append(pt)
                    # PV matmul for the PREVIOUS chunk: by the time the PE
                    # finishes s(j), exp(j-1) has drained, so the PE never
                    # stalls on the Activation engine
                    if j > 0:
                        nc.tensor.matmul(
                            ps_o[:], v_bf[:, 2 * p + hh, j - 1, :],
                            pts[j - 1][:],
                            start=(j - 1 == 0), stop=False,
                        )
                nc.tensor.matmul(
                    ps_o[:], v_bf[:, 2 * p + hh, nj - 1, :], pts[nj - 1][:],
                    start=False, stop=True,
                )
                pending.append((hh, qb, ps_o, den))
        flush_pending()


def _phase_e(nc, tc, p, ag_out, wout, y_part, y_out, groups):
    """Half output projection for pass p's heads (4 srcs x 2 heads).
    Pass-0 result is kept in the SBUF tile y_part; pass 1 adds onto it."""
    with (
        tc.tile_pool(name=f"ysb{p}", bufs=3) as ysbp,
        tc.tile_pool(name=f"a2asb{p}", bufs=1) as a2ap,
        tc.tile_pool(name=f"wo{p}", bufs=1) as wop,
        tc.tile_pool(name=f"psY{p}", bufs=3, space="PSUM") as psY,
    ):
        # all weights for this phase in two DMAs (dispatch cost on the Sync
        # sequencer is ~650ns per DMA instruction, so batch aggressively);
        # no dependence on the AllGather, so they stream during it
        wout_p = wout.rearrange("(s four) d c -> s four d c", four=4)[:, 2 * p:2 * p + 2]
        wt = wop.tile([128, 2, GROUP, NCH, 128], BF16, tag="wt", name="wt")
        for hh in range(2):
            nc.sync.dma_start(
                out=wt[:, hh].rearrange("d s ct c -> d s (ct c)"),
                in_=wout_p[:, hh].rearrange("s d c -> d s c"),
            )

        a2a_sb = a2ap.tile([128, GROUP, 2, QB], BF16, tag="a2asb", name="a2a_sb")
        rank = nc.sync.cc_rank(groups)
        # this core's QB-wide q slice of the gathered attT lives entirely in
        # AG chunk qb == rank; select it with a dynamic outer-dim offset
        agf = ag_out[p].rearrange("qb s h d q -> (qb s h) d q")
        for s in range(GROUP):
            for hh in range(2):
                nc.sync.dma_start(
                    out=a2a_sb[:, s, hh, :],
                    in_=agf[bass.ds(rank * (GROUP * 2) + (2 * s + hh), 1)],
                )
        for ct in range(NCH):
            ps_y = psY.tile([128, QB], F32, tag="y", name="ps_y")
            for s in range(GROUP):
                for hh in range(2):
                    nc.tensor.matmul(
                        ps_y[:], wt[:, hh, s, ct, :], a2a_sb[:, s, hh, :],
                        start=(s == 0 and hh == 0),
                        stop=(s == GROUP - 1 and hh == 1),
                    )
            if p == 0:
                nc.vector.tensor_copy(out=y_part[:, ct, :], in_=ps_y[:])
            else:
                y_sb = ysbp.tile([128, QB], F32, tag="ysb", name="ysb")
                nc.vector.tensor_tensor(
                    out=y_sb[:], in0=ps_y[:], in1=y_part[:, ct, :],
                    op=mybir.AluOpType.add)
                nc.sync.dma_start(
                    out=y_out[ct * 128:(ct + 1) * 128, :], in_=y_sb[:])


def build_nc():
    nc = bacc.Bacc(None, num_devices=N_CORES)

    # ---- DRAM parameters (per-core data) ----
    xT = nc.declare_dram_parameter("xT", [C, T], BF16, isOutput=False)
    wproj = nc.declare_dram_parameter("wproj", [C, 12, 128], BF16, isOutput=False)
    wout = nc.declare_dram_parameter("wout", [H, HD, C], BF16, isOutput=False)
    cos2 = nc.declare_dram_parameter("cos2", [128, T], F32R, isOutput=False)
    sins = nc.declare_dram_parameter("sins", [128, T], F32R, isOutput=False)
    masks = nc.declare_dram_parameter("masks", [128, 4, QB], BF16, isOutput=False)
    ones = nc.declare_dram_parameter("ones", [128, 1], F32R, isOutput=False)

    kT_out = nc.declare_dram_parameter("kT_out", [HPC, 128, T], F32R, isOutput=True)
    v_out = nc.declare_dram_parameter("v_out", [HPC, NKV, 128, HD], F32R, isOutput=True)
    y_out = nc.declare_dram_parameter("y_out", [C, QB], F32, isOutput=True)

    groups = [[0, 1, 2, 3], [4, 5, 6, 7]]

    with tile.TileContext(nc) as tc:
        with (
            tc.tile_pool(name="const", bufs=1) as constp,
            tc.tile_pool(name="dram", bufs=1, space="DRAM") as dramp,
        ):
            cos_sb = constp.tile([128, T], F32R, tag="cos", name="cos_sb")
            sin_sb = constp.tile([128, T], F32R, tag="sin", name="sin_sb")
            ones_sb = constp.tile([128, 1], F32R, tag="ones", name="ones_sb")
            # bf16 v for all 4 heads lives at outer scope: projected in
            # pass 0, consumed by both attention passes (the f32r copy for
            # the v output only needs to survive pass 0)
            v_bf = constp.tile([128, HPC, NKV, HD], BF16, tag="vallbf", name="v_bf")

            # per (pass, q-block) AllGather buffers (one contiguous tile
            # per pass so phase E can select chunk qb==rank dynamically)
            ag_in = [dramp.tile([NQB, 2, 128, QB], BF16, name=f"ag_in{p}")
                     for p in range(NPASS)]
            ag_out = [dramp.tile([NQB, GROUP, 2, 128, QB], BF16, name=f"ag_out{p}")
                      for p in range(NPASS)]

            xT_r = xT.rearrange("(ci p) t -> p ci t", p=128)
            wproj_r = wproj.rearrange("(ci p) j d -> p ci j d", p=128)
            # constants are needed ~40us in; issue their DMAs after the
            # first-pass weight/x DMAs so the first matmuls start sooner
            nc.sync.dma_start(out=cos_sb[:], in_=cos2[:])
            nc.sync.dma_start(out=sin_sb[:], in_=sins[:])
            nc.sync.dma_start(out=ones_sb[:], in_=ones[:])

            for p in range(NPASS):
                with (
                    tc.tile_pool(name=f"qkv{p}", bufs=1) as qkvp,
                    tc.tile_pool(name=f"scr{p}", bufs=2) as scrp,
                ):
                    qk_bf = _phase_b(
                        nc, tc, p, qkvp, scrp, v_bf, cos_sb, sin_sb,
                        wproj_r, xT_r, kT_out, v_out)
                    if p == 1:
                        # pass-0 output projection: AllGather #0 chunks have
                        # long since landed; runs on a warm PE before the
                        # pass-1 attention stream
                        ypp = tc.tile_pool(name="ypart", bufs=1)
                        ypool = ypp.__enter__()
                        y_part = ypool.tile([128, NCH, QB], F32, tag="ypart",
                                            name="y_part")
                        _phase_e(nc, tc, 0, ag_out, wout, y_part,
                                 y_out, groups)
                    _phase_c(nc, tc, p, qk_bf, v_bf, scrp, ones_sb, masks,
                             ag_in, ag_out, groups)
                    if p == 1:
                        _phase_e(nc, tc, 1, ag_out, wout, y_part, y_out,
                                 groups)
                        ypp.__exit__(None, None, None)

    nc.compile()
    return nc


def _host_prep(x, w_qkv, w_out, start_pos):
    x = np.asarray(x, dtype=np.float32)
    w_qkv = np.asarray(w_qkv, dtype=np.float32)
    w_out = np.asarray(w_out, dtype=np.float32)
    sp = int(start_pos)

    xT = [np.ascontiguousarray(x[b].T).astype(ml_dtypes.bfloat16) for b in range(B)]

    # RoPE tables in [hd, T] layout (halves stacked), sin with baked sign
    inv_freq = 1.0 / (ROPE_BASE ** (np.arange(0, HD, 2, dtype=np.float32) / HD))
    pos = (sp + np.arange(T)).astype(np.float32)
    ang = np.outer(pos, inv_freq)          # [T, 64]
    cosv = np.cos(ang).T.astype(np.float32)  # [64, T]
    sinv = np.sin(ang).T.astype(np.float32)
    cos2 = np.ascontiguousarray(np.concatenate([cosv, cosv], 0))    # [128, T]
    sins = np.ascontiguousarray(np.concatenate([-sinv, sinv], 0))   # [128, T]

    # causal 0/1 masks for the 4 diagonal chunk offsets: [kv,(r),q]
    kv = np.arange(128)[:, None]
    q = np.arange(QB)[None, :]
    masks = np.stack(
        [(128 * r + kv <= q).astype(ml_dtypes.bfloat16) for r in range(4)], axis=1)

    ones = np.ones((128, 1), np.float32)

    # per-core weight slices: wproj[c] = [C, pass, 6, 128]
    wq = w_qkv[:, 0 * C:1 * C].reshape(C, H, HD)
    wk = w_qkv[:, 1 * C:2 * C].reshape(C, H, HD)
    wv = w_qkv[:, 2 * C:3 * C].reshape(C, H, HD)
    wproj_cores = []
    for hg in range(GROUP):
        hs = [4 * hg + j for j in range(4)]
        slots = [wq[:, hs[0]], wq[:, hs[1]], wk[:, hs[0]], wk[:, hs[1]],
                 wv[:, hs[0]], wv[:, hs[1]], wv[:, hs[2]], wv[:, hs[3]],
                 wq[:, hs[2]], wq[:, hs[3]], wk[:, hs[2]], wk[:, hs[3]]]
        wproj_cores.append(np.ascontiguousarray(
            np.stack(slots, axis=1)).astype(ml_dtypes.bfloat16))  # [C, 12, 128]

    wout_r = np.ascontiguousarray(w_out.reshape(H, HD, C)).astype(ml_dtypes.bfloat16)

    in_maps = []
    for c in range(N_CORES):
        b, hg = c // GROUP, c % GROUP
        in_maps.append({
            "xT": xT[b],
            "wproj": wproj_cores[hg],
            "wout": wout_r,
            "cos2": cos2,
            "sins": sins,
            "masks": masks,
            "ones": ones,
        })
    return in_maps


def _assemble(results):
    y = np.empty((B, T, C), np.float32)
    k = np.empty((B, H, T, HD), np.float32)
    v = np.empty((B, H, T, HD), np.float32)
    for c in range(N_CORES):
        b, hg = c // GROUP, c % GROUP
        r = results[c]
        y[b, QB * hg:QB * (hg + 1), :] = r["y_out"].T
        for lh in range(HPC):
            hglob = 4 * hg + lh
            k[b, hglob] = r["kT_out"][lh].T
            v[b, hglob] = r["v_out"][lh].reshape(T, HD)
    return y, k, v


def run(x, w_qkv, w_out, start_pos, **spmd_kwargs):
    if "nc" not in _CACHED:
        _CACHED["nc"] = build_nc()
    nc = _CACHED["nc"]
    in_maps = _host_prep(x, w_qkv, w_out, start_pos)
    res = run_bass_kernel_spmd(nc, in_maps, list(range(N_CORES)), **spmd_kwargs)
    return _assemble(res.results), res


def kernel(x, w_qkv, w_out, start_pos):
    (y, k, v), _ = run(x, w_qkv, w_out, start_pos)
    return y, k, v
